# revision 13
# baseline (speedup 1.0000x reference)
"""DimeNet-style GNN message passing on 8 trn2 NeuronCores.

Self-contained: kernel(**inputs) -> np.ndarray [64, 128].

Distribution: nodes padded to 20480 = 8 cores x 20 blocks x 128; edges owned
by dst's core (sorted by dst), triplets owned by j_idx's core (sorted by
j_idx; triplets with j_idx >= N dropped -- they cannot affect the output
because agg_e is only ever read at node indices < N).

Per layer l:   m  = relu(h[k_idx] @ W1h + rc @ W1rc)          [triplets]
               A  = segsum_j(m);  V = A @ W2b                 [per node]
               fin= relu((h @ W2t)[src] + V[dst])             [edges]
               NA = segsum_dst(fin)
               h' = h @ Wnt + NA @ Wnb + b_n
Readout:       out = relu(relu(meanpool(h3)) @ Wo1 + bo1) @ Wo2 + bo2

Layer 1 needs no device gathers (host supplies x[src]^T / x[k_idx]^T tiles).
Layers 2/3 gather rows from an all-gathered [U|Hh] table with [128,1]
indirect DMAs; everything else moves via one-hot matmuls (dst-block local).
"""
import numpy as np
import ml_dtypes

import concourse.bass as bass
import concourse.mybir as mybir
import concourse.tile as tile
from concourse.tile import TileContext, ScopedClock
from concourse.bass_utils import run_bass_kernel_spmd

P = 128
H = 256
G = 64
OUT_C = 128
NCORES = 8
BLOCKS_PER_CORE = 20
NODES_PER_CORE = BLOCKS_PER_CORE * P   # 2560
NP_TOT = NCORES * NODES_PER_CORE       # 20480
BF = mybir.dt.bfloat16
F32 = mybir.dt.float32
I32 = mybir.dt.int32

_NEG = 0  # pad slot index


# ---------------------------------------------------------------------------
# Tile-framework compiler workarounds (this walrus build allows only ONE sync
# wait per instruction).
# ---------------------------------------------------------------------------
def _patched_drain_and_barrier(self, tick_clock, wait_clock):
    nc = self.nc
    drain_inst = nc.sync.drain()
    wait_clock.add_sem_waits(
        drain_inst.ins, ScopedClock({None: tick_clock.global_clock})
    )
    si = drain_inst.ins.sync_info
    if si is not None and si.on_wait is not None and len(si.on_wait) > 1:
        waits = list(si.on_wait)
        si.on_wait = waits[:1]
        for w in waits[1:]:
            d2 = nc.sync.drain()
            si2 = d2.ins.sync_info
            if si2 is None:
                d2.ins.sync_info = mybir.SyncInfo(on_wait=[w], on_update=[])
            else:
                si2.on_wait = [w]
    nc.all_engine_barrier()
    assert self.sems is not None
    popped = nc._tile_sem_poison_stack.pop()
    assert popped is self._sem_poison
    nc.clear_and_free_semaphores(list(self.sems.allocated().values()))
    nc.all_engine_barrier()


TileContext._drain_and_barrier = _patched_drain_and_barrier


def split_multi_waits(nc):
    for f in nc.m.functions:
        for bb in f.blocks:
            insts = list(bb.instructions)
            out = []
            changed = False
            for ins in insts:
                si = ins.sync_info
                if si is not None and si.on_wait is not None and len(si.on_wait) > 1:
                    waits = list(si.on_wait)
                    for i, w in enumerate(waits[:-1]):
                        nop = mybir.InstEventSemaphore(
                            name=f"{ins.name}_wsplit{i}", ins=[], outs=[]
                        )
                        nop.engine = ins.engine
                        nop.sync_info = mybir.SyncInfo(on_wait=[w], on_update=[])
                        nc.register_instruction(nop)
                        out.append(nop)
                        changed = True
                    si.on_wait = waits[-1:]
                out.append(ins)
            if changed:
                bb.instructions.clear()
                bb.instructions.extend(out)


# ---------------------------------------------------------------------------
# Host-side planning
# ---------------------------------------------------------------------------
def _plan(inputs):
    bf16 = ml_dtypes.bfloat16
    x = np.asarray(inputs["x"], np.float32)
    N = x.shape[0]
    edge_index = np.asarray(inputs["edge_index"], np.int64)
    rbf = np.asarray(inputs["rbf"], np.float32)
    cbf = np.asarray(inputs["cbf"], np.float32)
    k_idx = np.asarray(inputs["k_idx"], np.int64)
    j_idx = np.asarray(inputs["j_idx"], np.int64)
    batch = np.asarray(inputs["batch"], np.int64)
    src, dst = edge_index[0], edge_index[1]
    E = src.shape[0]

    # --- edges: owner core = dst // NODES_PER_CORE, sorted by dst ---
    e_order = np.argsort(dst, kind="stable")
    e_core = dst[e_order] // NODES_PER_CORE
    e_blk = (dst[e_order] % NODES_PER_CORE) // P  # block within core

    # per (core, block) edge lists
    edge_lists = [[None] * BLOCKS_PER_CORE for _ in range(NCORES)]
    for c in range(NCORES):
        in_c = e_order[e_core == c]
        blk = (dst[in_c] % NODES_PER_CORE) // P
        for b in range(BLOCKS_PER_CORE):
            edge_lists[c][b] = in_c[blk == b]
    # common per-block tile counts (max across cores)
    EBT = [
        max(1, max((len(edge_lists[c][b]) + P - 1) // P for c in range(NCORES)))
        for b in range(BLOCKS_PER_CORE)
    ]
    ET = sum(EBT)

    # --- triplets: keep j_idx < N, owner = j_idx // NODES_PER_CORE ---
    keep = j_idx < N
    tk = np.nonzero(keep)[0]
    tj = j_idx[tk]
    t_order = tk[np.argsort(tj, kind="stable")]
    trip_lists = [[None] * BLOCKS_PER_CORE for _ in range(NCORES)]
    for c in range(NCORES):
        sel = t_order[(j_idx[t_order] // NODES_PER_CORE) == c]
        blk = (j_idx[sel] % NODES_PER_CORE) // P
        for b in range(BLOCKS_PER_CORE):
            trip_lists[c][b] = sel[blk == b]
    TBT = [
        max(1, max((len(trip_lists[c][b]) + P - 1) // P for c in range(NCORES)))
        for b in range(BLOCKS_PER_CORE)
    ]
    TT = sum(TBT)

    # --- per-core arrays ---
    per_core = []
    # global mean-pool counts
    cnt = np.bincount(batch, minlength=G).astype(np.float32)
    cnt = np.maximum(cnt, 1.0)
    iota_row = np.tile(np.arange(P, dtype=np.float32), (P, 1)).astype(bf16)
    ident = np.eye(P, dtype=np.float32)

    for c in range(NCORES):
        esrc = np.zeros((ET, P), np.int64)
        eoff = np.zeros((ET, P), np.float32) + 255.0  # pad -> no one-hot match
        t0 = 0
        for b in range(BLOCKS_PER_CORE):
            el = edge_lists[c][b]
            nt = EBT[b]
            buf_src = np.zeros(nt * P, np.int64)
            buf_off = np.full(nt * P, 255.0, np.float32)
            buf_src[: len(el)] = src[el]
            buf_off[: len(el)] = (dst[el] % NODES_PER_CORE) % P
            esrc[t0 : t0 + nt] = buf_src.reshape(nt, P)
            eoff[t0 : t0 + nt] = buf_off.reshape(nt, P)
            t0 += nt
        # A_ne one-hot [ET, P(node), P(edge)] for V broadcast
        ane = (eoff[:, None, :] == np.arange(P, dtype=np.float32)[None, :, None])
        # x[src]^T tiles for layer 1: [ET, IN_C, P]
        xsrcT = np.ascontiguousarray(
            x[esrc.reshape(-1)].reshape(ET, P, x.shape[1]).transpose(0, 2, 1)
        )

        tkid = np.zeros((TT, P), np.int64)
        toff = np.zeros((TT, P), np.float32) + 255.0
        trc = np.zeros((TT, P, 12), np.float32)
        t0 = 0
        for b in range(BLOCKS_PER_CORE):
            tl = trip_lists[c][b]
            nt = TBT[b]
            bk = np.zeros(nt * P, np.int64)
            bo = np.full(nt * P, 255.0, np.float32)
            brc = np.zeros((nt * P, 12), np.float32)
            bk[: len(tl)] = k_idx[tl]
            bo[: len(tl)] = (j_idx[tl] % NODES_PER_CORE) % P
            brc[: len(tl), :6] = rbf[j_idx[tl]]
            brc[: len(tl), 6:] = cbf[tl]
            tkid[t0 : t0 + nt] = bk.reshape(nt, P)
            toff[t0 : t0 + nt] = bo.reshape(nt, P)
            trc[t0 : t0 + nt] = brc.reshape(nt, P, 12)
            t0 += nt
        trcT = np.ascontiguousarray(trc.transpose(0, 2, 1))  # [TT, 12, P]
        xkT = np.ascontiguousarray(
            x[tkid.reshape(-1)].reshape(TT, P, x.shape[1]).transpose(0, 2, 1)
        )

        # pooling matrix P_T [20, 128, 64] fp32: rows scaled by 1/cnt
        n0 = c * NODES_PER_CORE
        pt = np.zeros((BLOCKS_PER_CORE, P, G), np.float32)
        for b in range(BLOCKS_PER_CORE):
            for i in range(P):
                n = n0 + b * P + i
                if n < N:
                    pt[b, i, batch[n]] = 1.0 / cnt[batch[n]]

        # x^T for own nodes [IN_C, 2560]
        xo = np.zeros((NODES_PER_CORE, x.shape[1]), np.float32)
        hi = min(N, n0 + NODES_PER_CORE)
        if hi > n0:
            xo[: hi - n0] = x[n0:hi]
        xT_own = np.ascontiguousarray(xo.T)

        per_core.append(
            dict(
                e_src=esrc.T.astype(np.int32).copy(),        # [P, ET]
                e_off=eoff.T.astype(np.float32).copy(),            # [P, ET]
                e_ane=ane.astype(bf16).copy(),               # [ET, P, P]
                e_xsrcT=xsrcT.astype(bf16).copy(),           # [ET, IN_C, P]
                t_k=tkid.T.astype(np.int32).copy(),          # [P, TT]
                t_off=toff.T.astype(np.float32).copy(),            # [P, TT]
                t_rcT=trcT.astype(bf16).copy(),              # [TT, 12, P]
                t_xkT=xkT.astype(bf16).copy(),               # [TT, IN_C, P]
                poolT=pt,                                    # [20, P, G] f32
                xT_own=xT_own.astype(bf16).copy(),           # [IN_C, 2560]
            )
        )

    # --- weights (shared across cores) ---
    IN_C = x.shape[1]
    wb = {}
    for l in range(3):
        in_c = IN_C if l == 0 else H
        W1 = np.asarray(inputs[f"W_e1_{l}"], np.float32)  # [in_c+12, H]
        W2 = np.asarray(inputs[f"W_e2_{l}"], np.float32)  # [H+in_c, H]
        Wn = np.asarray(inputs[f"W_n_{l}"], np.float32)   # [in_c+H, H]
        bn = np.asarray(inputs[f"b_n_{l}"], np.float32)
        wb[f"w1h_{l}"] = W1[:in_c].reshape(in_c // P, P, H).astype(bf16)
        wb[f"w1rc_{l}"] = W1[in_c:].astype(bf16)                      # [12, H]
        wb[f"w2t_{l}"] = W2[:in_c].reshape(in_c // P, P, H).astype(bf16)
        wb[f"w2b_{l}"] = W2[in_c:].reshape(H // P, P, H).astype(bf16)
        wb[f"wn_{l}"] = Wn.reshape((in_c + H) // P, P, H).astype(bf16)
        wb[f"bn_{l}"] = bn.reshape(1, H).astype(bf16)
    wb["wo1"] = np.asarray(inputs["W_o1"], np.float32).reshape(H // P, P, H)
    wb["bo1"] = np.asarray(inputs["b_o1"], np.float32).reshape(1, H)
    wb["wo2"] = np.asarray(inputs["W_o2"], np.float32).reshape(H // P, P, OUT_C)
    wb["bo2"] = np.asarray(inputs["b_o2"], np.float32).reshape(1, OUT_C)
    wb["iota_row"] = np.asarray(iota_row)
    wb["ident_bf"] = ident.astype(bf16)
    wb["ones_bf"] = np.ones((1, P), bf16)
    wb["ones_f32"] = np.ones((1, G), np.float32)

    return dict(ET=ET, EBT=EBT, TT=TT, TBT=TBT, IN_C=IN_C,
                per_core=per_core, weights=wb)


# ---------------------------------------------------------------------------
# Device program
# ---------------------------------------------------------------------------
def _build(plan, n_cores=NCORES):
    ET, EBT, TT, TBT, IN_C = (
        plan["ET"], plan["EBT"], plan["TT"], plan["TBT"], plan["IN_C"]
    )
    nc = bass.Bass()

    # -- dram inputs --
    d = {}
    def din(name, shape, dt):
        d[name] = nc.dram_tensor(name, list(shape), dt, kind="ExternalInput")
        return d[name]

    din("e_src", (P, ET), I32)
    din("e_off", (P, ET), F32)
    din("e_ane", (ET, P, P), BF)
    din("e_xsrcT", (ET, IN_C, P), BF)
    din("t_k", (P, TT), I32)
    din("t_off", (P, TT), F32)
    din("t_rcT", (TT, 12, P), BF)
    din("t_xkT", (TT, IN_C, P), BF)
    din("poolT", (BLOCKS_PER_CORE, P, G), F32)
    din("xT_own", (IN_C, NODES_PER_CORE), BF)
    for l in range(3):
        in_c = IN_C if l == 0 else H
        din(f"w1h_{l}", (in_c // P, P, H), BF)
        din(f"w1rc_{l}", (12, H), BF)
        din(f"w2t_{l}", (in_c // P, P, H), BF)
        din(f"w2b_{l}", (H // P, P, H), BF)
        din(f"wn_{l}", ((in_c + H) // P, P, H), BF)
        din(f"bn_{l}", (1, H), BF)
    din("wo1", (H // P, P, H), F32)
    din("bo1", (1, H), F32)
    din("wo2", (H // P, P, OUT_C), F32)
    din("bo2", (1, OUT_C), F32)
    din("iota_row", (P, P), BF)
    din("ident_bf", (P, P), BF)
    din("ones_bf", (1, P), BF)
    din("ones_f32", (1, G), F32)

    out_ext = nc.dram_tensor("out", [G, OUT_C], F32, kind="ExternalOutput")
    import os as _os
    _dbg = bool(_os.environ.get("K_DBG"))
    if _dbg:
        dbg_h2 = nc.dram_tensor("dbg_h2", [P, 2 * NODES_PER_CORE], BF,
                                kind="ExternalOutput")
        dbg_at = nc.dram_tensor("dbg_at", [P, 2 * P], BF, kind="ExternalOutput")
        dbg_vd = nc.dram_tensor("dbg_vd", [P, H], BF, kind="ExternalOutput")
        dbg_fin = nc.dram_tensor("dbg_fin", [P, H], BF, kind="ExternalOutput")
        dbg_nat = nc.dram_tensor("dbg_nat", [P, 2 * P], BF, kind="ExternalOutput")
        dbg_m = nc.dram_tensor("dbg_m", [P, H], BF, kind="ExternalOutput")

    # internal DRAM
    ag_ins = [nc.dram_tensor(f"ag_in{i}", [NODES_PER_CORE, 2 * H], BF)
              for i in range(2)]
    tables = [
        nc.dram_tensor(f"table_{l}", [n_cores * NODES_PER_CORE, 2 * H], BF,
                       addr_space="Shared")
        for l in (1, 2)
    ]
    ar_in = nc.dram_tensor("ar_in", [G, H], F32)
    ar_out = nc.dram_tensor("ar_out", [G, H], F32, addr_space="Shared")
    groups = [list(range(n_cores))]

    with TileContext(nc) as tc:
        with (
            tc.tile_pool(name="const", bufs=1) as cs,
            tc.tile_pool(name="state", bufs=1) as st,
            tc.tile_pool(name="work", bufs=2) as wk,
            tc.tile_pool(name="psum1", bufs=1, space="PSUM") as ps1,
            tc.tile_pool(name="psum2", bufs=2, space="PSUM") as ps,
        ):
            iota = cs.tile([P, P], BF)
            nc.sync.dma_start(out=iota[:], in_=d["iota_row"][:, :])
            idb = cs.tile([P, P], BF)
            nc.sync.dma_start(out=idb[:], in_=d["ident_bf"][:, :])
            ones_bf = cs.tile([1, P], BF)
            nc.sync.dma_start(out=ones_bf[:], in_=d["ones_bf"][:, :])
            ones_f = cs.tile([1, G], F32)
            nc.sync.dma_start(out=ones_f[:], in_=d["ones_f32"][:, :])

            eoffs = cs.tile([P, ET], F32)
            nc.sync.dma_start(out=eoffs[:], in_=d["e_off"][:, :])
            toffs = cs.tile([P, TT], F32)
            nc.sync.dma_start(out=toffs[:], in_=d["t_off"][:, :])
            esrc = cs.tile([P, ET], I32)
            nc.sync.dma_start(out=esrc[:], in_=d["e_src"][:, :])
            tkix = cs.tile([P, TT], I32)
            nc.sync.dma_start(out=tkix[:], in_=d["t_k"][:, :])

            # weights resident
            W = {}
            for l in range(3):
                in_c = IN_C if l == 0 else H
                for nm, kc in ((f"w1h_{l}", in_c // P), (f"w2t_{l}", in_c // P),
                               (f"w2b_{l}", H // P), (f"wn_{l}", (in_c + H) // P)):
                    t = cs.tile([P, kc, H], BF, tag=nm)
                    nc.sync.dma_start(
                        out=t[:], in_=d[nm][:, :, :].rearrange("k p h -> p k h")
                    )
                    W[nm] = t
                t = cs.tile([12, H], BF, tag=f"w1rc_{l}")
                nc.sync.dma_start(out=t[:], in_=d[f"w1rc_{l}"][:, :])
                W[f"w1rc_{l}"] = t
                t = cs.tile([1, H], BF, tag=f"bn_{l}")
                nc.sync.dma_start(out=t[:], in_=d[f"bn_{l}"][:, :])
                W[f"bn_{l}"] = t

            # h state (transposed, own nodes) [P, kchunks, 2560]
            hT = st.tile([P, 2, NODES_PER_CORE], BF)
            nc.vector.memset(hT[:], 0.0)
            nc.sync.dma_start(
                out=hT[:, 0:IN_C // P, :],
                in_=d["xT_own"][:, :].rearrange("(k p) n -> p k n", p=P),
            )

            pooled_ps = ps1.tile([G, H], F32, space="PSUM", tag="pooled")

            for l in range(3):
                in_c = IN_C if l == 0 else H
                kc_in = in_c // P
                table = tables[l - 1] if l > 0 else None

                e_t0 = 0
                t_t0 = 0
                hT_new = st.tile([P, 2, NODES_PER_CORE], BF, tag=f"hTn{l % 2}")
                for b in range(BLOCKS_PER_CORE):
                    nbt = TBT[b]
                    # ---- triplet stage: A_T accumulation ----
                    at_ps0 = ps1.tile([P, P], F32, space="PSUM", tag="at0")
                    at_ps1 = ps1.tile([P, P], F32, space="PSUM", tag="at1")
                    at_ps = [at_ps0, at_ps1]
                    for tt in range(nbt):
                        ti = t_t0 + tt
                        rcT = wk.tile([12, P], BF, tag="rcT")
                        nc.sync.dma_start(out=rcT[:], in_=d["t_rcT"][ti, :, :])
                        m_ps = ps.tile([P, H], F32, space="PSUM", tag="work")
                        nc.tensor.matmul(
                            out=m_ps[:], lhsT=rcT[:], rhs=W[f"w1rc_{l}"][:],
                            start=True, stop=False,
                        )
                        if l == 0:
                            xk = wk.tile([P, kc_in, P], BF, tag="xk")
                            nc.sync.dma_start(
                                out=xk[:],
                                in_=d["t_xkT"][ti, :, :].rearrange(
                                    "(k p) e -> p k e", p=P),
                            )
                            for k in range(kc_in):
                                nc.tensor.matmul(
                                    out=m_ps[:], lhsT=xk[:, k, :],
                                    rhs=W[f"w1h_{l}"][:, k, :],
                                    start=False, stop=(k == kc_in - 1),
                                )
                        else:
                            hg = wk.tile([P, H], BF, tag="hg")
                            nc.gpsimd.indirect_dma_start(
                                out=hg[:], out_offset=None, in_=table[:, :],
                                in_offset=bass.IndirectOffsetOnAxis(
                                    ap=tkix[:, ti:ti + 1], axis=0),
                                element_offset=H,
                            )
                            nc.tensor.matmul(
                                out=m_ps[:], lhsT=idb[:], rhs=hg[:],
                                start=False, stop=True,
                            )
                        m_sb = wk.tile([P, H], BF, tag="msb")
                        nc.scalar.activation(
                            out=m_sb[:], in_=m_ps[:],
                            func=mybir.ActivationFunctionType.Relu,
                        )
                        if _dbg and l == 0 and b == 0 and tt == 0:
                            nc.sync.dma_start(out=dbg_m[:, :], in_=m_sb[:])
                        # B_en one-hot [t, n]
                        ben = wk.tile([P, P], BF, tag="ben")
                        nc.vector.tensor_scalar(
                            out=ben[:], in0=iota[:],
                            scalar1=toffs[:, ti:ti + 1], scalar2=None,
                            op0=mybir.AluOpType.is_equal,
                        )
                        for k in range(2):
                            nc.tensor.matmul(
                                out=at_ps[k][:], lhsT=m_sb[:, k * P:(k + 1) * P],
                                rhs=ben[:],
                                start=(tt == 0), stop=(tt == nbt - 1),
                            )
                    t_t0 += nbt
                    # A_T psum -> sbuf, V_D = A_T.T @ W2b
                    at_sb = wk.tile([P, 2, P], BF, tag="atsb")
                    for k in range(2):
                        nc.scalar.activation(
                            out=at_sb[:, k, :], in_=at_ps[k][:],
                            func=mybir.ActivationFunctionType.Copy,
                        )
                    if _dbg and l == 0 and b == 0:
                        nc.sync.dma_start(
                            out=dbg_at[:, :],
                            in_=at_sb[:, :, :].rearrange("p k n -> p (k n)"))
                    vd_ps = ps1.tile([P, H], F32, space="PSUM", tag="misc")
                    for k in range(2):
                        nc.tensor.matmul(
                            out=vd_ps[:], lhsT=at_sb[:, k, :],
                            rhs=W[f"w2b_{l}"][:, k, :],
                            start=(k == 0), stop=(k == 1),
                        )
                    vd = wk.tile([P, H], BF, tag="vdsb")
                    nc.scalar.activation(
                        out=vd[:], in_=vd_ps[:],
                        func=mybir.ActivationFunctionType.Copy,
                    )
                    if _dbg and l == 0 and b == 0:
                        nc.sync.dma_start(out=dbg_vd[:, :], in_=vd[:])

                    # ---- edge stage ----
                    nbe = EBT[b]
                    nat_ps0 = ps1.tile([P, P], F32, space="PSUM", tag="nat0")
                    nat_ps1 = ps1.tile([P, P], F32, space="PSUM", tag="nat1")
                    nat_ps = [nat_ps0, nat_ps1]
                    for et in range(nbe):
                        ei = e_t0 + et
                        fin_ps = ps.tile([P, H], F32, space="PSUM", tag="work")
                        # V broadcast: lhsT = A_ne (host), rhs = V_D
                        ane = wk.tile([P, P], BF, tag="ane")
                        nc.sync.dma_start(out=ane[:], in_=d["e_ane"][ei, :, :])
                        nc.tensor.matmul(
                            out=fin_ps[:], lhsT=ane[:], rhs=vd[:],
                            start=True, stop=False,
                        )
                        if l == 0:
                            xs = wk.tile([P, kc_in, P], BF, tag="xs")
                            nc.sync.dma_start(
                                out=xs[:],
                                in_=d["e_xsrcT"][ei, :, :].rearrange(
                                    "(k p) e -> p k e", p=P),
                            )
                            for k in range(kc_in):
                                nc.tensor.matmul(
                                    out=fin_ps[:], lhsT=xs[:, k, :],
                                    rhs=W[f"w2t_{l}"][:, k, :],
                                    start=False, stop=(k == kc_in - 1),
                                )
                        else:
                            ug = wk.tile([P, H], BF, tag="ug")
                            nc.gpsimd.indirect_dma_start(
                                out=ug[:], out_offset=None, in_=table[:, :],
                                in_offset=bass.IndirectOffsetOnAxis(
                                    ap=esrc[:, ei:ei + 1], axis=0),
                                element_offset=0,
                            )
                            nc.tensor.matmul(
                                out=fin_ps[:], lhsT=idb[:], rhs=ug[:],
                                start=False, stop=True,
                            )
                        fin = wk.tile([P, H], BF, tag="finsb")
                        nc.scalar.activation(
                            out=fin[:], in_=fin_ps[:],
                            func=mybir.ActivationFunctionType.Relu,
                        )
                        if _dbg and l == 0 and b == 0 and et == 0:
                            nc.sync.dma_start(out=dbg_fin[:, :], in_=fin[:])
                        aen = wk.tile([P, P], BF, tag="aen")
                        nc.vector.tensor_scalar(
                            out=aen[:], in0=iota[:],
                            scalar1=eoffs[:, ei:ei + 1], scalar2=None,
                            op0=mybir.AluOpType.is_equal,
                        )
                        for k in range(2):
                            nc.tensor.matmul(
                                out=nat_ps[k][:],
                                lhsT=fin[:, k * P:(k + 1) * P], rhs=aen[:],
                                start=(et == 0), stop=(et == nbe - 1),
                            )
                    e_t0 += nbe

                    nat_sb = wk.tile([P, 2, P], BF, tag="natsb")
                    for k in range(2):
                        nc.scalar.activation(
                            out=nat_sb[:, k, :], in_=nat_ps[k][:],
                            func=mybir.ActivationFunctionType.Copy,
                        )
                    if _dbg and l == 0 and b == 0:
                        nc.sync.dma_start(
                            out=dbg_nat[:, :],
                            in_=nat_sb[:, :, :].rearrange("p k n -> p (k n)"))

                    # ---- node update ----
                    hn_ps = ps1.tile([P, H], F32, space="PSUM", tag="misc")
                    nc.tensor.matmul(
                        out=hn_ps[:], lhsT=ones_bf[:],
                        rhs=W[f"bn_{l}"][:], start=True, stop=False,
                    )
                    for k in range(kc_in):
                        nc.tensor.matmul(
                            out=hn_ps[:],
                            lhsT=hT[:, k, b * P:(b + 1) * P],
                            rhs=W[f"wn_{l}"][:, k, :],
                            start=False, stop=False,
                        )
                    for k in range(2):
                        nc.tensor.matmul(
                            out=hn_ps[:], lhsT=nat_sb[:, k, :],
                            rhs=W[f"wn_{l}"][:, kc_in + k, :],
                            start=False, stop=(k == 1),
                        )

                    if l < 2:
                        hn_sb = wk.tile([P, H], BF, tag="hnsb")
                        nc.scalar.activation(
                            out=hn_sb[:], in_=hn_ps[:],
                            func=mybir.ActivationFunctionType.Copy,
                        )
                        # transpose to maintain hT_new
                        for k in range(2):
                            tr_ps = ps1.tile([P, P], BF, space="PSUM", tag="misc")
                            nc.tensor.transpose(
                                out=tr_ps[:], in_=hn_sb[:, k * P:(k + 1) * P],
                                identity=idb[:],
                            )
                            nc.scalar.activation(
                                out=hT_new[:, k, b * P:(b + 1) * P], in_=tr_ps[:],
                                func=mybir.ActivationFunctionType.Copy,
                            )
                        # next-layer table rows: U = h' @ w2t, Hh = h' @ w1h
                        for half, wname in ((0, f"w2t_{l + 1}"), (1, f"w1h_{l + 1}")):
                            tb_ps = ps1.tile([P, H], F32, space="PSUM", tag="misc")
                            for k in range(2):
                                nc.tensor.matmul(
                                    out=tb_ps[:],
                                    lhsT=hT_new[:, k, b * P:(b + 1) * P],
                                    rhs=W[wname][:, k, :],
                                    start=(k == 0), stop=(k == 1),
                                )
                            tb_sb = wk.tile([P, H], BF, tag="tbsb")
                            nc.scalar.activation(
                                out=tb_sb[:], in_=tb_ps[:],
                                func=mybir.ActivationFunctionType.Copy,
                            )
                            nc.sync.dma_start(
                                out=ag_ins[l][b * P:(b + 1) * P,
                                              half * H:(half + 1) * H],
                                in_=tb_sb[:],
                            )
                    else:
                        # layer 3: pooling contribution
                        h3 = wk.tile([P, H], F32, tag="h3")
                        nc.scalar.activation(
                            out=h3[:], in_=hn_ps[:],
                            func=mybir.ActivationFunctionType.Copy,
                        )
                        pt = wk.tile([P, G], F32, tag="pt")
                        nc.sync.dma_start(out=pt[:], in_=d["poolT"][b, :, :])
                        nc.tensor.matmul(
                            out=pooled_ps[:], lhsT=pt[:], rhs=h3[:],
                            start=(b == 0), stop=(b == BLOCKS_PER_CORE - 1),
                        )

                if l < 2:
                    hT = hT_new
                    if _dbg and l == 0:
                        nc.sync.dma_start(
                            out=dbg_h2[:, :],
                            in_=hT_new[:, :, :].rearrange("p k n -> p (k n)"),
                        )
                    nc.gpsimd.collective_compute(
                        "AllGather",
                        mybir.AluOpType.bypass,
                        replica_groups=groups,
                        ins=[ag_ins[l][:, :]],
                        outs=[tables[l][:, :]],
                    )

            # ---- readout ----
            pooled = wk.tile([G, H], F32, tag="pooled_sb")
            nc.scalar.activation(
                out=pooled[:], in_=pooled_ps[:],
                func=mybir.ActivationFunctionType.Copy,
            )
            nc.sync.dma_start(out=ar_in[:, :], in_=pooled[:])
            nc.gpsimd.collective_compute(
                "AllReduce", mybir.AluOpType.add, replica_groups=groups,
                ins=[ar_in[:, :]], outs=[ar_out[:, :]],
            )
            gsum = wk.tile([G, H], F32, tag="gsum")
            nc.sync.dma_start(out=gsum[:], in_=ar_out[:, :])
            # g = relu(pooled)
            nc.vector.tensor_scalar_max(gsum[:], gsum[:], 0.0)

            idf = cs.tile([P, P], F32)
            nc.vector.tensor_copy(idf[:], idb[:])

            wo1 = cs.tile([P, 2, H], F32)
            nc.sync.dma_start(
                out=wo1[:], in_=d["wo1"][:, :, :].rearrange("k p h -> p k h"))
            bo1 = cs.tile([1, H], F32)
            nc.sync.dma_start(out=bo1[:], in_=d["bo1"][:, :])
            wo2 = cs.tile([P, 2, OUT_C], F32)
            nc.sync.dma_start(
                out=wo2[:], in_=d["wo2"][:, :, :].rearrange("k p h -> p k h"))
            bo2 = cs.tile([1, OUT_C], F32)
            nc.sync.dma_start(out=bo2[:], in_=d["bo2"][:, :])

            def transpose_gx(src_tile, width):
                # [G, width] f32 -> [P, width//P, G] f32 (lhsT chunks)
                out_t = wk.tile([P, width // P, G], F32, tag="gxT")
                for k in range(width // P):
                    tp = ps1.tile([P, G], F32, space="PSUM", tag="misc")
                    nc.tensor.transpose(
                        out=tp[:], in_=src_tile[:, k * P:(k + 1) * P],
                        identity=idf[0:G, 0:G],
                    )
                    nc.scalar.activation(
                        out=out_t[:, k, :], in_=tp[:],
                        func=mybir.ActivationFunctionType.Copy,
                    )
                return out_t

            gT = transpose_gx(gsum, H)
            o1_ps = ps1.tile([G, H], F32, space="PSUM", tag="misc")
            nc.tensor.matmul(out=o1_ps[:], lhsT=ones_f[:], rhs=bo1[:],
                             start=True, stop=False)
            for k in range(2):
                nc.tensor.matmul(out=o1_ps[:], lhsT=gT[:, k, :],
                                 rhs=wo1[:, k, :], start=False, stop=(k == 1))
            o1 = wk.tile([G, H], F32, tag="o1sb")
            nc.scalar.activation(out=o1[:], in_=o1_ps[:],
                                 func=mybir.ActivationFunctionType.Relu)
            o1T = transpose_gx(o1, H)
            o2_ps = ps1.tile([G, OUT_C], F32, space="PSUM", tag="misc")
            nc.tensor.matmul(out=o2_ps[:], lhsT=ones_f[:], rhs=bo2[:],
                             start=True, stop=False)
            for k in range(2):
                nc.tensor.matmul(out=o2_ps[:], lhsT=o1T[:, k, :],
                                 rhs=wo2[:, k, :], start=False, stop=(k == 1))
            o2 = wk.tile([G, OUT_C], F32, tag="o2sb")
            nc.scalar.activation(out=o2[:], in_=o2_ps[:],
                                 func=mybir.ActivationFunctionType.Copy)
            nc.sync.dma_start(out=out_ext[:, :], in_=o2[:])

    split_multi_waits(nc)
    return nc


_CACHE = {}


def kernel(**inputs) -> np.ndarray:
    plan = _plan(inputs)
    nc = _build(plan)
    wb = plan["weights"]
    in_maps = []
    for c in range(NCORES):
        m = dict(plan["per_core"][c])
        m = {
            "e_src": m["e_src"], "e_off": m["e_off"], "e_ane": m["e_ane"],
            "e_xsrcT": m["e_xsrcT"], "t_k": m["t_k"], "t_off": m["t_off"],
            "t_rcT": m["t_rcT"], "t_xkT": m["t_xkT"], "poolT": m["poolT"],
            "xT_own": m["xT_own"],
        }
        for k, v in wb.items():
            if k in ("iota_row", "ident_bf", "ones_bf", "ones_f32"):
                continue
            m[k] = v
        m["iota_row"] = wb["iota_row"]
        m["ident_bf"] = wb["ident_bf"]
        m["ones_bf"] = wb["ones_bf"]
        m["ones_f32"] = wb["ones_f32"]
        in_maps.append(m)
    res = run_bass_kernel_spmd(nc, in_maps, core_ids=list(range(NCORES)))
    return np.asarray(res.results[0]["out"], np.float32)


def kernel_profiled(**inputs):
    """Returns (out, exec_ns_estimate). Tries neuron-profile trace; falls back
    to steady-state wall time of repeated NEFF executions."""
    import time as _time
    plan = _plan(inputs)
    nc = _build(plan)
    wb = plan["weights"]
    in_maps = []
    for c in range(NCORES):
        m = dict(plan["per_core"][c])
        m.update(wb)
        in_maps.append(m)
    try:
        res = run_bass_kernel_spmd(
            nc, in_maps, core_ids=list(range(NCORES)), trace=True
        )
    except Exception:
        res = run_bass_kernel_spmd(nc, in_maps, core_ids=list(range(NCORES)))
    out = np.asarray(res.results[0]["out"], np.float32)
    if res.exec_time_ns is not None:
        return out, res.exec_time_ns, "neuron-profile"
    # fallback: repeated executes of the cached NEFF (includes dispatch)
    times = []
    for _ in range(3):
        t0 = _time.perf_counter()
        run_bass_kernel_spmd(nc, in_maps, core_ids=list(range(NCORES)))
        times.append(_time.perf_counter() - t0)
    return out, int(min(times) * 1e9), "wall-clock upper bound"


if __name__ == "__main__":
    pass


# revision 14
# speedup vs baseline: 1.0798x; 1.0798x over previous
"""DimeNet-style GNN message passing on 8 trn2 NeuronCores.

Self-contained: kernel(**inputs) -> np.ndarray [64, 128].

Distribution: nodes padded to 20480 = 8 cores x 20 blocks x 128; edges owned
by dst's core (sorted by dst), triplets owned by j_idx's core (sorted by
j_idx; triplets with j_idx >= N dropped -- they cannot affect the output
because agg_e is only ever read at node indices < N).

Per layer l:   m  = relu(h[k_idx] @ W1h + rc @ W1rc)          [triplets]
               A  = segsum_j(m);  V = A @ W2b                 [per node]
               fin= relu((h @ W2t)[src] + V[dst])             [edges]
               NA = segsum_dst(fin)
               h' = h @ Wnt + NA @ Wnb + b_n
Readout:       out = relu(relu(meanpool(h3)) @ Wo1 + bo1) @ Wo2 + bo2

Layer 1 needs no device gathers (host supplies x[src]^T / x[k_idx]^T tiles).
Layers 2/3 gather rows from an all-gathered [U|Hh] table with [128,1]
indirect DMAs; everything else moves via one-hot matmuls (dst-block local).
"""
import numpy as np
import ml_dtypes

import concourse.bass as bass
import concourse.mybir as mybir
import concourse.tile as tile
from concourse.tile import TileContext, ScopedClock
from concourse.bass_utils import run_bass_kernel_spmd

P = 128
H = 256
G = 64
OUT_C = 128
NCORES = 8
BLOCKS_PER_CORE = 20
NODES_PER_CORE = BLOCKS_PER_CORE * P   # 2560
NP_TOT = NCORES * NODES_PER_CORE       # 20480
BF = mybir.dt.bfloat16
F32 = mybir.dt.float32
I32 = mybir.dt.int32

_NEG = 0  # pad slot index


# ---------------------------------------------------------------------------
# Tile-framework compiler workarounds (this walrus build allows only ONE sync
# wait per instruction).
# ---------------------------------------------------------------------------
def _patched_drain_and_barrier(self, tick_clock, wait_clock):
    nc = self.nc
    drain_inst = nc.sync.drain()
    wait_clock.add_sem_waits(
        drain_inst.ins, ScopedClock({None: tick_clock.global_clock})
    )
    si = drain_inst.ins.sync_info
    if si is not None and si.on_wait is not None and len(si.on_wait) > 1:
        waits = list(si.on_wait)
        si.on_wait = waits[:1]
        for w in waits[1:]:
            d2 = nc.sync.drain()
            si2 = d2.ins.sync_info
            if si2 is None:
                d2.ins.sync_info = mybir.SyncInfo(on_wait=[w], on_update=[])
            else:
                si2.on_wait = [w]
    nc.all_engine_barrier()
    assert self.sems is not None
    popped = nc._tile_sem_poison_stack.pop()
    assert popped is self._sem_poison
    nc.clear_and_free_semaphores(list(self.sems.allocated().values()))
    nc.all_engine_barrier()


TileContext._drain_and_barrier = _patched_drain_and_barrier


def split_multi_waits(nc):
    for f in nc.m.functions:
        for bb in f.blocks:
            insts = list(bb.instructions)
            out = []
            changed = False
            for ins in insts:
                si = ins.sync_info
                if si is not None and si.on_wait is not None and len(si.on_wait) > 1:
                    waits = list(si.on_wait)
                    for i, w in enumerate(waits[:-1]):
                        nop = mybir.InstEventSemaphore(
                            name=f"{ins.name}_wsplit{i}", ins=[], outs=[]
                        )
                        nop.engine = ins.engine
                        nop.sync_info = mybir.SyncInfo(on_wait=[w], on_update=[])
                        nc.register_instruction(nop)
                        out.append(nop)
                        changed = True
                    si.on_wait = waits[-1:]
                out.append(ins)
            if changed:
                bb.instructions.clear()
                bb.instructions.extend(out)


# ---------------------------------------------------------------------------
# Host-side planning
# ---------------------------------------------------------------------------
def _plan(inputs):
    bf16 = ml_dtypes.bfloat16
    x = np.asarray(inputs["x"], np.float32)
    N = x.shape[0]
    edge_index = np.asarray(inputs["edge_index"], np.int64)
    rbf = np.asarray(inputs["rbf"], np.float32)
    cbf = np.asarray(inputs["cbf"], np.float32)
    k_idx = np.asarray(inputs["k_idx"], np.int64)
    j_idx = np.asarray(inputs["j_idx"], np.int64)
    batch = np.asarray(inputs["batch"], np.int64)
    src, dst = edge_index[0], edge_index[1]
    E = src.shape[0]

    # --- edges: owner core = dst // NODES_PER_CORE, sorted by dst ---
    e_order = np.argsort(dst, kind="stable")
    e_core = dst[e_order] // NODES_PER_CORE
    e_blk = (dst[e_order] % NODES_PER_CORE) // P  # block within core

    # per (core, block) edge lists
    edge_lists = [[None] * BLOCKS_PER_CORE for _ in range(NCORES)]
    for c in range(NCORES):
        in_c = e_order[e_core == c]
        blk = (dst[in_c] % NODES_PER_CORE) // P
        for b in range(BLOCKS_PER_CORE):
            edge_lists[c][b] = in_c[blk == b]
    # common per-block tile counts (max across cores)
    EBT = [
        max(1, max((len(edge_lists[c][b]) + P - 1) // P for c in range(NCORES)))
        for b in range(BLOCKS_PER_CORE)
    ]
    ET = sum(EBT)

    # --- triplets: keep j_idx < N, owner = j_idx // NODES_PER_CORE ---
    keep = j_idx < N
    tk = np.nonzero(keep)[0]
    tj = j_idx[tk]
    t_order = tk[np.argsort(tj, kind="stable")]
    trip_lists = [[None] * BLOCKS_PER_CORE for _ in range(NCORES)]
    for c in range(NCORES):
        sel = t_order[(j_idx[t_order] // NODES_PER_CORE) == c]
        blk = (j_idx[sel] % NODES_PER_CORE) // P
        for b in range(BLOCKS_PER_CORE):
            trip_lists[c][b] = sel[blk == b]
    TBT = [
        max(1, max((len(trip_lists[c][b]) + P - 1) // P for c in range(NCORES)))
        for b in range(BLOCKS_PER_CORE)
    ]
    TT = sum(TBT)

    # --- per-core arrays ---
    per_core = []
    # global mean-pool counts
    cnt = np.bincount(batch, minlength=G).astype(np.float32)
    cnt = np.maximum(cnt, 1.0)
    iota_row = np.tile(np.arange(P, dtype=np.float32), (P, 1)).astype(bf16)
    ident = np.eye(P, dtype=np.float32)

    for c in range(NCORES):
        esrc = np.zeros((ET, P), np.int64)
        eoff = np.zeros((ET, P), np.float32) + 255.0  # pad -> no one-hot match
        t0 = 0
        for b in range(BLOCKS_PER_CORE):
            el = edge_lists[c][b]
            nt = EBT[b]
            buf_src = np.zeros(nt * P, np.int64)
            buf_off = np.full(nt * P, 255.0, np.float32)
            buf_src[: len(el)] = src[el]
            buf_off[: len(el)] = (dst[el] % NODES_PER_CORE) % P
            esrc[t0 : t0 + nt] = buf_src.reshape(nt, P)
            eoff[t0 : t0 + nt] = buf_off.reshape(nt, P)
            t0 += nt
        # A_ne one-hot [ET, P(node), P(edge)] for V broadcast
        ane = (eoff[:, None, :] == np.arange(P, dtype=np.float32)[None, :, None])
        # x[src]^T tiles for layer 1: [ET, IN_C, P]
        xsrcT = np.ascontiguousarray(
            x[esrc.reshape(-1)].reshape(ET, P, x.shape[1]).transpose(0, 2, 1)
        )

        tkid = np.zeros((TT, P), np.int64)
        toff = np.zeros((TT, P), np.float32) + 255.0
        trc = np.zeros((TT, P, 12), np.float32)
        t0 = 0
        for b in range(BLOCKS_PER_CORE):
            tl = trip_lists[c][b]
            nt = TBT[b]
            bk = np.zeros(nt * P, np.int64)
            bo = np.full(nt * P, 255.0, np.float32)
            brc = np.zeros((nt * P, 12), np.float32)
            bk[: len(tl)] = k_idx[tl]
            bo[: len(tl)] = (j_idx[tl] % NODES_PER_CORE) % P
            brc[: len(tl), :6] = rbf[j_idx[tl]]
            brc[: len(tl), 6:] = cbf[tl]
            tkid[t0 : t0 + nt] = bk.reshape(nt, P)
            toff[t0 : t0 + nt] = bo.reshape(nt, P)
            trc[t0 : t0 + nt] = brc.reshape(nt, P, 12)
            t0 += nt
        trcT = np.ascontiguousarray(trc.transpose(0, 2, 1))  # [TT, 12, P]
        xkT = np.ascontiguousarray(
            x[tkid.reshape(-1)].reshape(TT, P, x.shape[1]).transpose(0, 2, 1)
        )

        # pooling matrix P_T [20, 128, 64] fp32: rows scaled by 1/cnt
        n0 = c * NODES_PER_CORE
        pt = np.zeros((BLOCKS_PER_CORE, P, G), np.float32)
        for b in range(BLOCKS_PER_CORE):
            for i in range(P):
                n = n0 + b * P + i
                if n < N:
                    pt[b, i, batch[n]] = 1.0 / cnt[batch[n]]

        # x^T for own nodes [IN_C, 2560]
        xo = np.zeros((NODES_PER_CORE, x.shape[1]), np.float32)
        hi = min(N, n0 + NODES_PER_CORE)
        if hi > n0:
            xo[: hi - n0] = x[n0:hi]
        xT_own = np.ascontiguousarray(xo.T)

        per_core.append(
            dict(
                e_src=esrc.T.astype(np.int32).copy(),        # [P, ET]
                e_off=eoff.T.astype(np.float32).copy(),            # [P, ET]
                e_ane=ane.astype(bf16).copy(),               # [ET, P, P]
                e_xsrcT=xsrcT.astype(bf16).copy(),           # [ET, IN_C, P]
                t_k=tkid.T.astype(np.int32).copy(),          # [P, TT]
                t_off=toff.T.astype(np.float32).copy(),            # [P, TT]
                t_rcT=trcT.astype(bf16).copy(),              # [TT, 12, P]
                t_xkT=xkT.astype(bf16).copy(),               # [TT, IN_C, P]
                poolT=pt,                                    # [20, P, G] f32
                xT_own=xT_own.astype(bf16).copy(),           # [IN_C, 2560]
            )
        )

    # --- weights (shared across cores) ---
    IN_C = x.shape[1]
    wb = {}
    for l in range(3):
        in_c = IN_C if l == 0 else H
        W1 = np.asarray(inputs[f"W_e1_{l}"], np.float32)  # [in_c+12, H]
        W2 = np.asarray(inputs[f"W_e2_{l}"], np.float32)  # [H+in_c, H]
        Wn = np.asarray(inputs[f"W_n_{l}"], np.float32)   # [in_c+H, H]
        bn = np.asarray(inputs[f"b_n_{l}"], np.float32)
        wb[f"w1h_{l}"] = W1[:in_c].reshape(in_c // P, P, H).astype(bf16)
        wb[f"w1rc_{l}"] = W1[in_c:].astype(bf16)                      # [12, H]
        wb[f"w2t_{l}"] = W2[:in_c].reshape(in_c // P, P, H).astype(bf16)
        wb[f"w2b_{l}"] = W2[in_c:].reshape(H // P, P, H).astype(bf16)
        wb[f"wn_{l}"] = Wn.reshape((in_c + H) // P, P, H).astype(bf16)
        wb[f"bn_{l}"] = bn.reshape(1, H).astype(bf16)
    wb["wo1"] = np.asarray(inputs["W_o1"], np.float32).reshape(H // P, P, H)
    wb["bo1"] = np.asarray(inputs["b_o1"], np.float32).reshape(1, H)
    wb["wo2"] = np.asarray(inputs["W_o2"], np.float32).reshape(H // P, P, OUT_C)
    wb["bo2"] = np.asarray(inputs["b_o2"], np.float32).reshape(1, OUT_C)
    wb["iota_row"] = np.asarray(iota_row)
    wb["ident_bf"] = ident.astype(bf16)
    wb["ones_bf"] = np.ones((1, P), bf16)
    wb["ones_f32"] = np.ones((1, G), np.float32)

    return dict(ET=ET, EBT=EBT, TT=TT, TBT=TBT, IN_C=IN_C,
                per_core=per_core, weights=wb)


# ---------------------------------------------------------------------------
# Device program
# ---------------------------------------------------------------------------
def _build(plan, n_cores=NCORES):
    ET, EBT, TT, TBT, IN_C = (
        plan["ET"], plan["EBT"], plan["TT"], plan["TBT"], plan["IN_C"]
    )
    nc = bass.Bass()

    # -- dram inputs --
    d = {}
    def din(name, shape, dt):
        d[name] = nc.dram_tensor(name, list(shape), dt, kind="ExternalInput")
        return d[name]

    din("e_src", (P, ET), I32)
    din("e_off", (P, ET), F32)
    din("e_xsrcT", (ET, IN_C, P), BF)
    din("t_k", (P, TT), I32)
    din("t_off", (P, TT), F32)
    din("t_rcT", (TT, 12, P), BF)
    din("t_xkT", (TT, IN_C, P), BF)
    din("poolT", (BLOCKS_PER_CORE, P, G), F32)
    din("xT_own", (IN_C, NODES_PER_CORE), BF)
    for l in range(3):
        in_c = IN_C if l == 0 else H
        din(f"w1h_{l}", (in_c // P, P, H), BF)
        din(f"w1rc_{l}", (12, H), BF)
        din(f"w2t_{l}", (in_c // P, P, H), BF)
        din(f"w2b_{l}", (H // P, P, H), BF)
        din(f"wn_{l}", ((in_c + H) // P, P, H), BF)
        din(f"bn_{l}", (1, H), BF)
    din("wo1", (H // P, P, H), F32)
    din("bo1", (1, H), F32)
    din("wo2", (H // P, P, OUT_C), F32)
    din("bo2", (1, OUT_C), F32)
    din("iota_row", (P, P), BF)
    din("ident_bf", (P, P), BF)
    din("ones_bf", (1, P), BF)
    din("ones_f32", (1, G), F32)

    out_ext = nc.dram_tensor("out", [G, OUT_C], F32, kind="ExternalOutput")
    import os as _os
    _dbg = bool(_os.environ.get("K_DBG"))
    if _dbg:
        dbg_h2 = nc.dram_tensor("dbg_h2", [P, 2 * NODES_PER_CORE], BF,
                                kind="ExternalOutput")
        dbg_at = nc.dram_tensor("dbg_at", [P, 2 * P], BF, kind="ExternalOutput")
        dbg_vd = nc.dram_tensor("dbg_vd", [P, H], BF, kind="ExternalOutput")
        dbg_fin = nc.dram_tensor("dbg_fin", [P, H], BF, kind="ExternalOutput")
        dbg_nat = nc.dram_tensor("dbg_nat", [P, 2 * P], BF, kind="ExternalOutput")
        dbg_m = nc.dram_tensor("dbg_m", [P, H], BF, kind="ExternalOutput")

    # internal DRAM
    ag_ins = [nc.dram_tensor(f"ag_in{i}", [NODES_PER_CORE, 2 * H], BF)
              for i in range(2)]
    tables = [
        nc.dram_tensor(f"table_{l}", [n_cores * NODES_PER_CORE, 2 * H], BF,
                       addr_space="Shared")
        for l in (1, 2)
    ]
    ar_in = nc.dram_tensor("ar_in", [G, H], F32)
    ar_out = nc.dram_tensor("ar_out", [G, H], F32, addr_space="Shared")
    groups = [list(range(n_cores))]

    with TileContext(nc) as tc:
        with (
            tc.tile_pool(name="const", bufs=1) as cs,
            tc.tile_pool(name="state", bufs=1) as st,
            tc.tile_pool(name="work", bufs=2) as wk,
            tc.tile_pool(name="psum1", bufs=1, space="PSUM") as ps1,
            tc.tile_pool(name="psum2", bufs=2, space="PSUM") as ps,
        ):
            iota = cs.tile([P, P], BF)
            nc.sync.dma_start(out=iota[:], in_=d["iota_row"][:, :])
            idb = cs.tile([P, P], BF)
            nc.sync.dma_start(out=idb[:], in_=d["ident_bf"][:, :])
            ones_bf = cs.tile([1, P], BF)
            nc.sync.dma_start(out=ones_bf[:], in_=d["ones_bf"][:, :])
            ones_f = cs.tile([1, G], F32)
            nc.sync.dma_start(out=ones_f[:], in_=d["ones_f32"][:, :])

            eoffs = cs.tile([P, ET], F32)
            nc.sync.dma_start(out=eoffs[:], in_=d["e_off"][:, :])
            toffs = cs.tile([P, TT], F32)
            nc.sync.dma_start(out=toffs[:], in_=d["t_off"][:, :])
            esrc = cs.tile([P, ET], I32)
            nc.sync.dma_start(out=esrc[:], in_=d["e_src"][:, :])
            tkix = cs.tile([P, TT], I32)
            nc.sync.dma_start(out=tkix[:], in_=d["t_k"][:, :])

            # weights resident
            W = {}
            for l in range(3):
                in_c = IN_C if l == 0 else H
                for nm, kc in ((f"w1h_{l}", in_c // P), (f"w2t_{l}", in_c // P),
                               (f"w2b_{l}", H // P), (f"wn_{l}", (in_c + H) // P)):
                    t = cs.tile([P, kc, H], BF, tag=nm)
                    nc.sync.dma_start(
                        out=t[:], in_=d[nm][:, :, :].rearrange("k p h -> p k h")
                    )
                    W[nm] = t
                t = cs.tile([12, H], BF, tag=f"w1rc_{l}")
                nc.sync.dma_start(out=t[:], in_=d[f"w1rc_{l}"][:, :])
                W[f"w1rc_{l}"] = t
                t = cs.tile([1, H], BF, tag=f"bn_{l}")
                nc.sync.dma_start(out=t[:], in_=d[f"bn_{l}"][:, :])
                W[f"bn_{l}"] = t

            # h state (transposed, own nodes) [P, kchunks, 2560]
            hT = st.tile([P, 2, NODES_PER_CORE], BF)
            nc.vector.memset(hT[:], 0.0)
            nc.sync.dma_start(
                out=hT[:, 0:IN_C // P, :],
                in_=d["xT_own"][:, :].rearrange("(k p) n -> p k n", p=P),
            )

            pooled_ps = ps1.tile([G, H], F32, space="PSUM", tag="pooled")

            for l in range(3):
                in_c = IN_C if l == 0 else H
                kc_in = in_c // P
                table = tables[l - 1] if l > 0 else None

                e_t0 = 0
                t_t0 = 0
                hT_new = st.tile([P, 2, NODES_PER_CORE], BF, tag=f"hTn{l % 2}")
                for b in range(BLOCKS_PER_CORE):
                    nbt = TBT[b]
                    # ---- triplet stage: A_T accumulation ----
                    at_ps0 = ps1.tile([P, P], F32, space="PSUM", tag="at0")
                    at_ps1 = ps1.tile([P, P], F32, space="PSUM", tag="at1")
                    at_ps = [at_ps0, at_ps1]
                    for tt in range(nbt):
                        ti = t_t0 + tt
                        rcT = wk.tile([12, P], BF, tag="rcT")
                        nc.sync.dma_start(out=rcT[:], in_=d["t_rcT"][ti, :, :])
                        m_ps = ps.tile([P, H], F32, space="PSUM", tag="work")
                        nc.tensor.matmul(
                            out=m_ps[:], lhsT=rcT[:], rhs=W[f"w1rc_{l}"][:],
                            start=True, stop=False,
                        )
                        if l == 0:
                            xk = wk.tile([P, kc_in, P], BF, tag="xk")
                            nc.sync.dma_start(
                                out=xk[:],
                                in_=d["t_xkT"][ti, :, :].rearrange(
                                    "(k p) e -> p k e", p=P),
                            )
                            for k in range(kc_in):
                                nc.tensor.matmul(
                                    out=m_ps[:], lhsT=xk[:, k, :],
                                    rhs=W[f"w1h_{l}"][:, k, :],
                                    start=False, stop=(k == kc_in - 1),
                                )
                        else:
                            hg = wk.tile([P, H], BF, tag="hg")
                            nc.gpsimd.indirect_dma_start(
                                out=hg[:], out_offset=None, in_=table[:, :],
                                in_offset=bass.IndirectOffsetOnAxis(
                                    ap=tkix[:, ti:ti + 1], axis=0),
                                element_offset=H,
                            )
                            nc.tensor.matmul(
                                out=m_ps[:], lhsT=idb[:], rhs=hg[:],
                                start=False, stop=True,
                            )
                        m_sb = wk.tile([P, H], BF, tag="msb")
                        nc.scalar.activation(
                            out=m_sb[:], in_=m_ps[:],
                            func=mybir.ActivationFunctionType.Relu,
                        )
                        if _dbg and l == 0 and b == 0 and tt == 0:
                            nc.sync.dma_start(out=dbg_m[:, :], in_=m_sb[:])
                        # B_en one-hot [t, n]
                        ben = wk.tile([P, P], BF, tag="ben")
                        nc.vector.tensor_scalar(
                            out=ben[:], in0=iota[:],
                            scalar1=toffs[:, ti:ti + 1], scalar2=None,
                            op0=mybir.AluOpType.is_equal,
                        )
                        for k in range(2):
                            nc.tensor.matmul(
                                out=at_ps[k][:], lhsT=m_sb[:, k * P:(k + 1) * P],
                                rhs=ben[:],
                                start=(tt == 0), stop=(tt == nbt - 1),
                            )
                    t_t0 += nbt
                    # A_T psum -> sbuf, V_D = A_T.T @ W2b
                    at_sb = wk.tile([P, 2, P], BF, tag="atsb")
                    for k in range(2):
                        nc.scalar.activation(
                            out=at_sb[:, k, :], in_=at_ps[k][:],
                            func=mybir.ActivationFunctionType.Copy,
                        )
                    if _dbg and l == 0 and b == 0:
                        nc.sync.dma_start(
                            out=dbg_at[:, :],
                            in_=at_sb[:, :, :].rearrange("p k n -> p (k n)"))
                    vd_ps = ps1.tile([P, H], F32, space="PSUM", tag="misc")
                    for k in range(2):
                        nc.tensor.matmul(
                            out=vd_ps[:], lhsT=at_sb[:, k, :],
                            rhs=W[f"w2b_{l}"][:, k, :],
                            start=(k == 0), stop=(k == 1),
                        )
                    vd = wk.tile([P, H], BF, tag="vdsb")
                    nc.scalar.activation(
                        out=vd[:], in_=vd_ps[:],
                        func=mybir.ActivationFunctionType.Copy,
                    )
                    if _dbg and l == 0 and b == 0:
                        nc.sync.dma_start(out=dbg_vd[:, :], in_=vd[:])

                    # ---- edge stage ----
                    nbe = EBT[b]
                    nat_ps0 = ps1.tile([P, P], F32, space="PSUM", tag="nat0")
                    nat_ps1 = ps1.tile([P, P], F32, space="PSUM", tag="nat1")
                    nat_ps = [nat_ps0, nat_ps1]
                    for et in range(nbe):
                        ei = e_t0 + et
                        fin_ps = ps.tile([P, H], F32, space="PSUM", tag="work")
                        # A_en one-hot, then PE-transpose -> A_ne for V bcast
                        aen = wk.tile([P, P], BF, tag="aen")
                        nc.vector.tensor_scalar(
                            out=aen[:], in0=iota[:],
                            scalar1=eoffs[:, ei:ei + 1], scalar2=None,
                            op0=mybir.AluOpType.is_equal,
                        )
                        ane_ps = ps.tile([P, P], BF, space="PSUM", tag="work")
                        nc.tensor.transpose(
                            out=ane_ps[:], in_=aen[:], identity=idb[:])
                        ane = wk.tile([P, P], BF, tag="ane")
                        nc.scalar.activation(
                            out=ane[:], in_=ane_ps[:],
                            func=mybir.ActivationFunctionType.Copy,
                        )
                        nc.tensor.matmul(
                            out=fin_ps[:], lhsT=ane[:], rhs=vd[:],
                            start=True, stop=False,
                        )
                        if l == 0:
                            xs = wk.tile([P, kc_in, P], BF, tag="xs")
                            nc.sync.dma_start(
                                out=xs[:],
                                in_=d["e_xsrcT"][ei, :, :].rearrange(
                                    "(k p) e -> p k e", p=P),
                            )
                            for k in range(kc_in):
                                nc.tensor.matmul(
                                    out=fin_ps[:], lhsT=xs[:, k, :],
                                    rhs=W[f"w2t_{l}"][:, k, :],
                                    start=False, stop=(k == kc_in - 1),
                                )
                        else:
                            ug = wk.tile([P, H], BF, tag="ug")
                            nc.gpsimd.indirect_dma_start(
                                out=ug[:], out_offset=None, in_=table[:, :],
                                in_offset=bass.IndirectOffsetOnAxis(
                                    ap=esrc[:, ei:ei + 1], axis=0),
                                element_offset=0,
                            )
                            nc.tensor.matmul(
                                out=fin_ps[:], lhsT=idb[:], rhs=ug[:],
                                start=False, stop=True,
                            )
                        fin = wk.tile([P, H], BF, tag="finsb")
                        nc.scalar.activation(
                            out=fin[:], in_=fin_ps[:],
                            func=mybir.ActivationFunctionType.Relu,
                        )
                        if _dbg and l == 0 and b == 0 and et == 0:
                            nc.sync.dma_start(out=dbg_fin[:, :], in_=fin[:])
                        for k in range(2):
                            nc.tensor.matmul(
                                out=nat_ps[k][:],
                                lhsT=fin[:, k * P:(k + 1) * P], rhs=aen[:],
                                start=(et == 0), stop=(et == nbe - 1),
                            )
                    e_t0 += nbe

                    nat_sb = wk.tile([P, 2, P], BF, tag="natsb")
                    for k in range(2):
                        nc.scalar.activation(
                            out=nat_sb[:, k, :], in_=nat_ps[k][:],
                            func=mybir.ActivationFunctionType.Copy,
                        )
                    if _dbg and l == 0 and b == 0:
                        nc.sync.dma_start(
                            out=dbg_nat[:, :],
                            in_=nat_sb[:, :, :].rearrange("p k n -> p (k n)"))

                    # ---- node update ----
                    hn_ps = ps1.tile([P, H], F32, space="PSUM", tag="misc")
                    nc.tensor.matmul(
                        out=hn_ps[:], lhsT=ones_bf[:],
                        rhs=W[f"bn_{l}"][:], start=True, stop=False,
                    )
                    for k in range(kc_in):
                        nc.tensor.matmul(
                            out=hn_ps[:],
                            lhsT=hT[:, k, b * P:(b + 1) * P],
                            rhs=W[f"wn_{l}"][:, k, :],
                            start=False, stop=False,
                        )
                    for k in range(2):
                        nc.tensor.matmul(
                            out=hn_ps[:], lhsT=nat_sb[:, k, :],
                            rhs=W[f"wn_{l}"][:, kc_in + k, :],
                            start=False, stop=(k == 1),
                        )

                    if l < 2:
                        hn_sb = wk.tile([P, H], BF, tag="hnsb")
                        nc.scalar.activation(
                            out=hn_sb[:], in_=hn_ps[:],
                            func=mybir.ActivationFunctionType.Copy,
                        )
                        # transpose to maintain hT_new
                        for k in range(2):
                            tr_ps = ps1.tile([P, P], BF, space="PSUM", tag="misc")
                            nc.tensor.transpose(
                                out=tr_ps[:], in_=hn_sb[:, k * P:(k + 1) * P],
                                identity=idb[:],
                            )
                            nc.scalar.activation(
                                out=hT_new[:, k, b * P:(b + 1) * P], in_=tr_ps[:],
                                func=mybir.ActivationFunctionType.Copy,
                            )
                        # next-layer table rows: U = h' @ w2t, Hh = h' @ w1h
                        for half, wname in ((0, f"w2t_{l + 1}"), (1, f"w1h_{l + 1}")):
                            tb_ps = ps1.tile([P, H], F32, space="PSUM", tag="misc")
                            for k in range(2):
                                nc.tensor.matmul(
                                    out=tb_ps[:],
                                    lhsT=hT_new[:, k, b * P:(b + 1) * P],
                                    rhs=W[wname][:, k, :],
                                    start=(k == 0), stop=(k == 1),
                                )
                            tb_sb = wk.tile([P, H], BF, tag="tbsb")
                            nc.scalar.activation(
                                out=tb_sb[:], in_=tb_ps[:],
                                func=mybir.ActivationFunctionType.Copy,
                            )
                            nc.sync.dma_start(
                                out=ag_ins[l][b * P:(b + 1) * P,
                                              half * H:(half + 1) * H],
                                in_=tb_sb[:],
                            )
                    else:
                        # layer 3: pooling contribution
                        h3 = wk.tile([P, H], F32, tag="h3")
                        nc.scalar.activation(
                            out=h3[:], in_=hn_ps[:],
                            func=mybir.ActivationFunctionType.Copy,
                        )
                        pt = wk.tile([P, G], F32, tag="pt")
                        nc.sync.dma_start(out=pt[:], in_=d["poolT"][b, :, :])
                        nc.tensor.matmul(
                            out=pooled_ps[:], lhsT=pt[:], rhs=h3[:],
                            start=(b == 0), stop=(b == BLOCKS_PER_CORE - 1),
                        )

                if l < 2:
                    hT = hT_new
                    if _dbg and l == 0:
                        nc.sync.dma_start(
                            out=dbg_h2[:, :],
                            in_=hT_new[:, :, :].rearrange("p k n -> p (k n)"),
                        )
                    nc.gpsimd.collective_compute(
                        "AllGather",
                        mybir.AluOpType.bypass,
                        replica_groups=groups,
                        ins=[ag_ins[l][:, :]],
                        outs=[tables[l][:, :]],
                    )

            # ---- readout ----
            pooled = wk.tile([G, H], F32, tag="pooled_sb")
            nc.scalar.activation(
                out=pooled[:], in_=pooled_ps[:],
                func=mybir.ActivationFunctionType.Copy,
            )
            nc.sync.dma_start(out=ar_in[:, :], in_=pooled[:])
            nc.gpsimd.collective_compute(
                "AllReduce", mybir.AluOpType.add, replica_groups=groups,
                ins=[ar_in[:, :]], outs=[ar_out[:, :]],
            )
            gsum = wk.tile([G, H], F32, tag="gsum")
            nc.sync.dma_start(out=gsum[:], in_=ar_out[:, :])
            # g = relu(pooled)
            nc.vector.tensor_scalar_max(gsum[:], gsum[:], 0.0)

            idf = cs.tile([P, P], F32)
            nc.vector.tensor_copy(idf[:], idb[:])

            wo1 = cs.tile([P, 2, H], F32)
            nc.sync.dma_start(
                out=wo1[:], in_=d["wo1"][:, :, :].rearrange("k p h -> p k h"))
            bo1 = cs.tile([1, H], F32)
            nc.sync.dma_start(out=bo1[:], in_=d["bo1"][:, :])
            wo2 = cs.tile([P, 2, OUT_C], F32)
            nc.sync.dma_start(
                out=wo2[:], in_=d["wo2"][:, :, :].rearrange("k p h -> p k h"))
            bo2 = cs.tile([1, OUT_C], F32)
            nc.sync.dma_start(out=bo2[:], in_=d["bo2"][:, :])

            def transpose_gx(src_tile, width):
                # [G, width] f32 -> [P, width//P, G] f32 (lhsT chunks)
                out_t = wk.tile([P, width // P, G], F32, tag="gxT")
                for k in range(width // P):
                    tp = ps1.tile([P, G], F32, space="PSUM", tag="misc")
                    nc.tensor.transpose(
                        out=tp[:], in_=src_tile[:, k * P:(k + 1) * P],
                        identity=idf[0:G, 0:G],
                    )
                    nc.scalar.activation(
                        out=out_t[:, k, :], in_=tp[:],
                        func=mybir.ActivationFunctionType.Copy,
                    )
                return out_t

            gT = transpose_gx(gsum, H)
            o1_ps = ps1.tile([G, H], F32, space="PSUM", tag="misc")
            nc.tensor.matmul(out=o1_ps[:], lhsT=ones_f[:], rhs=bo1[:],
                             start=True, stop=False)
            for k in range(2):
                nc.tensor.matmul(out=o1_ps[:], lhsT=gT[:, k, :],
                                 rhs=wo1[:, k, :], start=False, stop=(k == 1))
            o1 = wk.tile([G, H], F32, tag="o1sb")
            nc.scalar.activation(out=o1[:], in_=o1_ps[:],
                                 func=mybir.ActivationFunctionType.Relu)
            o1T = transpose_gx(o1, H)
            o2_ps = ps1.tile([G, OUT_C], F32, space="PSUM", tag="misc")
            nc.tensor.matmul(out=o2_ps[:], lhsT=ones_f[:], rhs=bo2[:],
                             start=True, stop=False)
            for k in range(2):
                nc.tensor.matmul(out=o2_ps[:], lhsT=o1T[:, k, :],
                                 rhs=wo2[:, k, :], start=False, stop=(k == 1))
            o2 = wk.tile([G, OUT_C], F32, tag="o2sb")
            nc.scalar.activation(out=o2[:], in_=o2_ps[:],
                                 func=mybir.ActivationFunctionType.Copy)
            nc.sync.dma_start(out=out_ext[:, :], in_=o2[:])

    split_multi_waits(nc)
    return nc


_CACHE = {}


def kernel(**inputs) -> np.ndarray:
    plan = _plan(inputs)
    nc = _build(plan)
    wb = plan["weights"]
    in_maps = []
    for c in range(NCORES):
        m = dict(plan["per_core"][c])
        m = {
            "e_src": m["e_src"], "e_off": m["e_off"],
            "e_xsrcT": m["e_xsrcT"], "t_k": m["t_k"], "t_off": m["t_off"],
            "t_rcT": m["t_rcT"], "t_xkT": m["t_xkT"], "poolT": m["poolT"],
            "xT_own": m["xT_own"],
        }
        for k, v in wb.items():
            if k in ("iota_row", "ident_bf", "ones_bf", "ones_f32"):
                continue
            m[k] = v
        m["iota_row"] = wb["iota_row"]
        m["ident_bf"] = wb["ident_bf"]
        m["ones_bf"] = wb["ones_bf"]
        m["ones_f32"] = wb["ones_f32"]
        in_maps.append(m)
    res = run_bass_kernel_spmd(nc, in_maps, core_ids=list(range(NCORES)))
    return np.asarray(res.results[0]["out"], np.float32)


def kernel_profiled(**inputs):
    """Returns (out, exec_ns_estimate). Tries neuron-profile trace; falls back
    to steady-state wall time of repeated NEFF executions."""
    import time as _time
    plan = _plan(inputs)
    nc = _build(plan)
    wb = plan["weights"]
    in_maps = []
    for c in range(NCORES):
        m = dict(plan["per_core"][c])
        m.update(wb)
        in_maps.append(m)
    try:
        res = run_bass_kernel_spmd(
            nc, in_maps, core_ids=list(range(NCORES)), trace=True
        )
    except Exception:
        res = run_bass_kernel_spmd(nc, in_maps, core_ids=list(range(NCORES)))
    out = np.asarray(res.results[0]["out"], np.float32)
    if res.exec_time_ns is not None:
        return out, res.exec_time_ns, "neuron-profile"
    # fallback: repeated executes of the cached NEFF (includes dispatch)
    times = []
    for _ in range(3):
        t0 = _time.perf_counter()
        run_bass_kernel_spmd(nc, in_maps, core_ids=list(range(NCORES)))
        times.append(_time.perf_counter() - t0)
    return out, int(min(times) * 1e9), "wall-clock upper bound"


if __name__ == "__main__":
    pass


# revision 16
# speedup vs baseline: 1.1864x; 1.0987x over previous
"""DimeNet-style GNN message passing on 8 trn2 NeuronCores.

Self-contained: kernel(**inputs) -> np.ndarray [64, 128].

Distribution: nodes padded to 20480 = 8 cores x 20 blocks x 128; edges owned
by dst's core (sorted by dst), triplets owned by j_idx's core (sorted by
j_idx; triplets with j_idx >= N dropped -- they cannot affect the output
because agg_e is only ever read at node indices < N).

Per layer l:   m  = relu(h[k_idx] @ W1h + rc @ W1rc)          [triplets]
               A  = segsum_j(m);  V = A @ W2b                 [per node]
               fin= relu((h @ W2t)[src] + V[dst])             [edges]
               NA = segsum_dst(fin)
               h' = h @ Wnt + NA @ Wnb + b_n
Readout:       out = relu(relu(meanpool(h3)) @ Wo1 + bo1) @ Wo2 + bo2

Layer 1 needs no device gathers (host supplies x[src]^T / x[k_idx]^T tiles).
Layers 2/3 gather rows from an all-gathered [U|Hh] table with [128,1]
indirect DMAs; everything else moves via one-hot matmuls (dst-block local).
"""
import numpy as np
import ml_dtypes

import concourse.bass as bass
import concourse.mybir as mybir
import concourse.tile as tile
from concourse.tile import TileContext, ScopedClock
from concourse.bass_utils import run_bass_kernel_spmd

P = 128
H = 256
G = 64
OUT_C = 128
NCORES = 8
BLOCKS_PER_CORE = 20
NODES_PER_CORE = BLOCKS_PER_CORE * P   # 2560
NP_TOT = NCORES * NODES_PER_CORE       # 20480
BF = mybir.dt.bfloat16
F32 = mybir.dt.float32
I32 = mybir.dt.int32

_NEG = 0  # pad slot index


# ---------------------------------------------------------------------------
# Tile-framework compiler workarounds (this walrus build allows only ONE sync
# wait per instruction).
# ---------------------------------------------------------------------------
def _patched_drain_and_barrier(self, tick_clock, wait_clock):
    nc = self.nc
    drain_inst = nc.sync.drain()
    wait_clock.add_sem_waits(
        drain_inst.ins, ScopedClock({None: tick_clock.global_clock})
    )
    si = drain_inst.ins.sync_info
    if si is not None and si.on_wait is not None and len(si.on_wait) > 1:
        waits = list(si.on_wait)
        si.on_wait = waits[:1]
        for w in waits[1:]:
            d2 = nc.sync.drain()
            si2 = d2.ins.sync_info
            if si2 is None:
                d2.ins.sync_info = mybir.SyncInfo(on_wait=[w], on_update=[])
            else:
                si2.on_wait = [w]
    nc.all_engine_barrier()
    assert self.sems is not None
    popped = nc._tile_sem_poison_stack.pop()
    assert popped is self._sem_poison
    nc.clear_and_free_semaphores(list(self.sems.allocated().values()))
    nc.all_engine_barrier()


TileContext._drain_and_barrier = _patched_drain_and_barrier


def split_multi_waits(nc):
    for f in nc.m.functions:
        for bb in f.blocks:
            insts = list(bb.instructions)
            out = []
            changed = False
            for ins in insts:
                si = ins.sync_info
                if si is not None and si.on_wait is not None and len(si.on_wait) > 1:
                    waits = list(si.on_wait)
                    for i, w in enumerate(waits[:-1]):
                        nop = mybir.InstEventSemaphore(
                            name=f"{ins.name}_wsplit{i}", ins=[], outs=[]
                        )
                        nop.engine = ins.engine
                        nop.sync_info = mybir.SyncInfo(on_wait=[w], on_update=[])
                        nc.register_instruction(nop)
                        out.append(nop)
                        changed = True
                    si.on_wait = waits[-1:]
                out.append(ins)
            if changed:
                bb.instructions.clear()
                bb.instructions.extend(out)


# ---------------------------------------------------------------------------
# Host-side planning
# ---------------------------------------------------------------------------
def _plan(inputs):
    bf16 = ml_dtypes.bfloat16
    x = np.asarray(inputs["x"], np.float32)
    N = x.shape[0]
    edge_index = np.asarray(inputs["edge_index"], np.int64)
    rbf = np.asarray(inputs["rbf"], np.float32)
    cbf = np.asarray(inputs["cbf"], np.float32)
    k_idx = np.asarray(inputs["k_idx"], np.int64)
    j_idx = np.asarray(inputs["j_idx"], np.int64)
    batch = np.asarray(inputs["batch"], np.int64)
    src, dst = edge_index[0], edge_index[1]
    E = src.shape[0]

    # --- edges: owner core = dst // NODES_PER_CORE, sorted by dst ---
    e_order = np.argsort(dst, kind="stable")
    e_core = dst[e_order] // NODES_PER_CORE
    e_blk = (dst[e_order] % NODES_PER_CORE) // P  # block within core

    # per (core, block) edge lists
    edge_lists = [[None] * BLOCKS_PER_CORE for _ in range(NCORES)]
    for c in range(NCORES):
        in_c = e_order[e_core == c]
        blk = (dst[in_c] % NODES_PER_CORE) // P
        for b in range(BLOCKS_PER_CORE):
            edge_lists[c][b] = in_c[blk == b]
    # common per-block tile counts (max across cores)
    EBT = [
        max(1, max((len(edge_lists[c][b]) + P - 1) // P for c in range(NCORES)))
        for b in range(BLOCKS_PER_CORE)
    ]
    ET = sum(EBT)

    # --- triplets: keep j_idx < N, owner = j_idx // NODES_PER_CORE ---
    keep = j_idx < N
    tk = np.nonzero(keep)[0]
    tj = j_idx[tk]
    t_order = tk[np.argsort(tj, kind="stable")]
    trip_lists = [[None] * BLOCKS_PER_CORE for _ in range(NCORES)]
    for c in range(NCORES):
        sel = t_order[(j_idx[t_order] // NODES_PER_CORE) == c]
        blk = (j_idx[sel] % NODES_PER_CORE) // P
        for b in range(BLOCKS_PER_CORE):
            trip_lists[c][b] = sel[blk == b]
    TBT = [
        max(1, max((len(trip_lists[c][b]) + P - 1) // P for c in range(NCORES)))
        for b in range(BLOCKS_PER_CORE)
    ]
    TT = sum(TBT)

    # --- per-core arrays ---
    per_core = []
    # global mean-pool counts
    cnt = np.bincount(batch, minlength=G).astype(np.float32)
    cnt = np.maximum(cnt, 1.0)
    iota_row = np.tile(np.arange(P, dtype=np.float32), (P, 1)).astype(bf16)
    ident = np.eye(P, dtype=np.float32)

    for c in range(NCORES):
        esrc = np.zeros((ET, P), np.int64)
        eoff = np.zeros((ET, P), np.float32) + 255.0  # pad -> no one-hot match
        t0 = 0
        for b in range(BLOCKS_PER_CORE):
            el = edge_lists[c][b]
            nt = EBT[b]
            buf_src = np.zeros(nt * P, np.int64)
            buf_off = np.full(nt * P, 255.0, np.float32)
            buf_src[: len(el)] = src[el]
            buf_off[: len(el)] = (dst[el] % NODES_PER_CORE) % P
            esrc[t0 : t0 + nt] = buf_src.reshape(nt, P)
            eoff[t0 : t0 + nt] = buf_off.reshape(nt, P)
            t0 += nt
        # x[src]^T tiles for layer 1: [ET, IN_C, P]
        xsrcT = np.ascontiguousarray(
            x[esrc.reshape(-1)].reshape(ET, P, x.shape[1]).transpose(0, 2, 1)
        )

        tkid = np.zeros((TT, P), np.int64)
        toff = np.zeros((TT, P), np.float32) + 255.0
        trc = np.zeros((TT, P, 12), np.float32)
        t0 = 0
        for b in range(BLOCKS_PER_CORE):
            tl = trip_lists[c][b]
            nt = TBT[b]
            bk = np.zeros(nt * P, np.int64)
            bo = np.full(nt * P, 255.0, np.float32)
            brc = np.zeros((nt * P, 12), np.float32)
            bk[: len(tl)] = k_idx[tl]
            bo[: len(tl)] = (j_idx[tl] % NODES_PER_CORE) % P
            brc[: len(tl), :6] = rbf[j_idx[tl]]
            brc[: len(tl), 6:] = cbf[tl]
            tkid[t0 : t0 + nt] = bk.reshape(nt, P)
            toff[t0 : t0 + nt] = bo.reshape(nt, P)
            trc[t0 : t0 + nt] = brc.reshape(nt, P, 12)
            t0 += nt
        trcT = np.ascontiguousarray(trc.transpose(0, 2, 1))  # [TT, 12, P]
        xkT = np.ascontiguousarray(
            x[tkid.reshape(-1)].reshape(TT, P, x.shape[1]).transpose(0, 2, 1)
        )

        # pooling matrix P_T [20, 128, 64] fp32: rows scaled by 1/cnt
        n0 = c * NODES_PER_CORE
        pt = np.zeros((BLOCKS_PER_CORE, P, G), np.float32)
        for b in range(BLOCKS_PER_CORE):
            for i in range(P):
                n = n0 + b * P + i
                if n < N:
                    pt[b, i, batch[n]] = 1.0 / cnt[batch[n]]

        # x^T for own nodes [IN_C, 2560]
        xo = np.zeros((NODES_PER_CORE, x.shape[1]), np.float32)
        hi = min(N, n0 + NODES_PER_CORE)
        if hi > n0:
            xo[: hi - n0] = x[n0:hi]
        xT_own = np.ascontiguousarray(xo.T)

        per_core.append(
            dict(
                e_src=esrc.T.astype(np.int32).copy(),        # [P, ET]
                e_off=eoff.T.astype(np.float32).copy(),            # [P, ET]
                e_xsrcT=xsrcT.astype(bf16).copy(),           # [ET, IN_C, P]
                t_k=tkid.T.astype(np.int32).copy(),          # [P, TT]
                t_off=toff.T.astype(np.float32).copy(),            # [P, TT]
                t_rcT=trcT.astype(bf16).copy(),              # [TT, 12, P]
                t_xkT=xkT.astype(bf16).copy(),               # [TT, IN_C, P]
                poolT=pt,                                    # [20, P, G] f32
                xT_own=xT_own.astype(bf16).copy(),           # [IN_C, 2560]
            )
        )

    # --- weights (shared across cores) ---
    IN_C = x.shape[1]
    wb = {}
    for l in range(3):
        in_c = IN_C if l == 0 else H
        W1 = np.asarray(inputs[f"W_e1_{l}"], np.float32)  # [in_c+12, H]
        W2 = np.asarray(inputs[f"W_e2_{l}"], np.float32)  # [H+in_c, H]
        Wn = np.asarray(inputs[f"W_n_{l}"], np.float32)   # [in_c+H, H]
        bn = np.asarray(inputs[f"b_n_{l}"], np.float32)
        wb[f"w1h_{l}"] = W1[:in_c].reshape(in_c // P, P, H).astype(bf16)
        wb[f"w1rc_{l}"] = W1[in_c:].astype(bf16)                      # [12, H]
        wb[f"w2t_{l}"] = W2[:in_c].reshape(in_c // P, P, H).astype(bf16)
        wb[f"w2b_{l}"] = W2[in_c:].reshape(H // P, P, H).astype(bf16)
        wb[f"wn_{l}"] = Wn.reshape((in_c + H) // P, P, H).astype(bf16)
        wb[f"bn_{l}"] = bn.reshape(1, H).astype(bf16)
    wb["wo1"] = np.asarray(inputs["W_o1"], np.float32).reshape(H // P, P, H)
    wb["bo1"] = np.asarray(inputs["b_o1"], np.float32).reshape(1, H)
    wb["wo2"] = np.asarray(inputs["W_o2"], np.float32).reshape(H // P, P, OUT_C)
    wb["bo2"] = np.asarray(inputs["b_o2"], np.float32).reshape(1, OUT_C)
    wb["iota_row"] = np.asarray(iota_row)
    wb["ident_bf"] = ident.astype(bf16)
    wb["ones_bf"] = np.ones((1, P), bf16)
    wb["ones_f32"] = np.ones((1, G), np.float32)

    return dict(ET=ET, EBT=EBT, TT=TT, TBT=TBT, IN_C=IN_C,
                per_core=per_core, weights=wb)


# ---------------------------------------------------------------------------
# Device program
# ---------------------------------------------------------------------------
def _build(plan, n_cores=NCORES):
    ET, EBT, TT, TBT, IN_C = (
        plan["ET"], plan["EBT"], plan["TT"], plan["TBT"], plan["IN_C"]
    )
    nc = bass.Bass()

    # -- dram inputs --
    d = {}
    def din(name, shape, dt):
        d[name] = nc.dram_tensor(name, list(shape), dt, kind="ExternalInput")
        return d[name]

    din("e_src", (P, ET), I32)
    din("e_off", (P, ET), F32)
    din("e_xsrcT", (ET, IN_C, P), BF)
    din("t_k", (P, TT), I32)
    din("t_off", (P, TT), F32)
    din("t_rcT", (TT, 12, P), BF)
    din("t_xkT", (TT, IN_C, P), BF)
    din("poolT", (BLOCKS_PER_CORE, P, G), F32)
    din("xT_own", (IN_C, NODES_PER_CORE), BF)
    for l in range(3):
        in_c = IN_C if l == 0 else H
        din(f"w1h_{l}", (in_c // P, P, H), BF)
        din(f"w1rc_{l}", (12, H), BF)
        din(f"w2t_{l}", (in_c // P, P, H), BF)
        din(f"w2b_{l}", (H // P, P, H), BF)
        din(f"wn_{l}", ((in_c + H) // P, P, H), BF)
        din(f"bn_{l}", (1, H), BF)
    din("wo1", (H // P, P, H), F32)
    din("bo1", (1, H), F32)
    din("wo2", (H // P, P, OUT_C), F32)
    din("bo2", (1, OUT_C), F32)
    din("iota_row", (P, P), BF)
    din("ident_bf", (P, P), BF)
    din("ones_bf", (1, P), BF)
    din("ones_f32", (1, G), F32)

    out_ext = nc.dram_tensor("out", [G, OUT_C], F32, kind="ExternalOutput")
    import os as _os
    _dbg = bool(_os.environ.get("K_DBG"))
    if _dbg:
        dbg_h2 = nc.dram_tensor("dbg_h2", [P, 2 * NODES_PER_CORE], BF,
                                kind="ExternalOutput")
        dbg_at = nc.dram_tensor("dbg_at", [P, 2 * P], BF, kind="ExternalOutput")
        dbg_vd = nc.dram_tensor("dbg_vd", [P, H], BF, kind="ExternalOutput")
        dbg_fin = nc.dram_tensor("dbg_fin", [P, H], BF, kind="ExternalOutput")
        dbg_nat = nc.dram_tensor("dbg_nat", [P, 2 * P], BF, kind="ExternalOutput")
        dbg_m = nc.dram_tensor("dbg_m", [P, H], BF, kind="ExternalOutput")

    # internal DRAM
    ag_ins = [nc.dram_tensor(f"ag_in{i}", [NODES_PER_CORE, 2 * H], BF)
              for i in range(2)]
    tables = [
        nc.dram_tensor(f"table_{l}", [n_cores * NODES_PER_CORE, 2 * H], BF,
                       addr_space="Shared")
        for l in (1, 2)
    ]
    ar_in = nc.dram_tensor("ar_in", [G, H], F32)
    ar_out = nc.dram_tensor("ar_out", [G, H], F32, addr_space="Shared")
    groups = [list(range(n_cores))]

    with TileContext(nc) as tc:
        with (
            tc.tile_pool(name="const", bufs=1) as cs,
            tc.tile_pool(name="state", bufs=1) as st,
            tc.tile_pool(name="work", bufs=4) as wk,
            tc.tile_pool(name="psum1", bufs=1, space="PSUM") as ps1,
            tc.tile_pool(name="psum2", bufs=2, space="PSUM") as ps,
        ):
            iota = cs.tile([P, P], BF)
            nc.sync.dma_start(out=iota[:], in_=d["iota_row"][:, :])
            idb = cs.tile([P, P], BF)
            nc.sync.dma_start(out=idb[:], in_=d["ident_bf"][:, :])
            ones_bf = cs.tile([1, P], BF)
            nc.sync.dma_start(out=ones_bf[:], in_=d["ones_bf"][:, :])
            ones_f = cs.tile([1, G], F32)
            nc.sync.dma_start(out=ones_f[:], in_=d["ones_f32"][:, :])

            eoffs = cs.tile([P, ET], F32)
            nc.sync.dma_start(out=eoffs[:], in_=d["e_off"][:, :])
            toffs = cs.tile([P, TT], F32)
            nc.sync.dma_start(out=toffs[:], in_=d["t_off"][:, :])
            esrc = cs.tile([P, ET], I32)
            nc.sync.dma_start(out=esrc[:], in_=d["e_src"][:, :])
            tkix = cs.tile([P, TT], I32)
            nc.sync.dma_start(out=tkix[:], in_=d["t_k"][:, :])

            # weights resident
            W = {}
            for l in range(3):
                in_c = IN_C if l == 0 else H
                for nm, kc in ((f"w1h_{l}", in_c // P), (f"w2t_{l}", in_c // P),
                               (f"w2b_{l}", H // P), (f"wn_{l}", (in_c + H) // P)):
                    t = cs.tile([P, kc, H], BF, tag=nm)
                    nc.sync.dma_start(
                        out=t[:], in_=d[nm][:, :, :].rearrange("k p h -> p k h")
                    )
                    W[nm] = t
                t = cs.tile([12, H], BF, tag=f"w1rc_{l}")
                nc.sync.dma_start(out=t[:], in_=d[f"w1rc_{l}"][:, :])
                W[f"w1rc_{l}"] = t
                t = cs.tile([1, H], BF, tag=f"bn_{l}")
                nc.sync.dma_start(out=t[:], in_=d[f"bn_{l}"][:, :])
                W[f"bn_{l}"] = t

            # h state (transposed, own nodes) [P, kchunks, 2560]
            hT = st.tile([P, 2, NODES_PER_CORE], BF)
            nc.vector.memset(hT[:], 0.0)
            nc.sync.dma_start(
                out=hT[:, 0:IN_C // P, :],
                in_=d["xT_own"][:, :].rearrange("(k p) n -> p k n", p=P),
            )

            pooled_ps = ps1.tile([G, H], F32, space="PSUM", tag="pooled")

            for l in range(3):
                in_c = IN_C if l == 0 else H
                kc_in = in_c // P
                table = tables[l - 1] if l > 0 else None

                e_t0 = 0
                t_t0 = 0
                hT_new = st.tile([P, 2, NODES_PER_CORE], BF, tag=f"hTn{l % 2}")
                for b in range(BLOCKS_PER_CORE):
                    nbt = TBT[b]
                    # ---- triplet stage: A_T accumulation ----
                    at_ps0 = ps1.tile([P, P], F32, space="PSUM", tag="at0")
                    at_ps1 = ps1.tile([P, P], F32, space="PSUM", tag="at1")
                    at_ps = [at_ps0, at_ps1]
                    for tt in range(nbt):
                        ti = t_t0 + tt
                        rcT = wk.tile([12, P], BF, tag="rcT")
                        nc.sync.dma_start(out=rcT[:], in_=d["t_rcT"][ti, :, :])
                        m_ps = ps.tile([P, H], F32, space="PSUM", tag="work")
                        nc.tensor.matmul(
                            out=m_ps[:], lhsT=rcT[:], rhs=W[f"w1rc_{l}"][:],
                            start=True, stop=False,
                        )
                        if l == 0:
                            xk = wk.tile([P, kc_in, P], BF, tag="xk")
                            nc.sync.dma_start(
                                out=xk[:],
                                in_=d["t_xkT"][ti, :, :].rearrange(
                                    "(k p) e -> p k e", p=P),
                            )
                            for k in range(kc_in):
                                nc.tensor.matmul(
                                    out=m_ps[:], lhsT=xk[:, k, :],
                                    rhs=W[f"w1h_{l}"][:, k, :],
                                    start=False, stop=(k == kc_in - 1),
                                )
                        else:
                            hg = wk.tile([P, H], BF, tag="hg")
                            nc.gpsimd.indirect_dma_start(
                                out=hg[:], out_offset=None, in_=table[:, :],
                                in_offset=bass.IndirectOffsetOnAxis(
                                    ap=tkix[:, ti:ti + 1], axis=0),
                                element_offset=H,
                            )
                            nc.tensor.matmul(
                                out=m_ps[:], lhsT=idb[:], rhs=hg[:],
                                start=False, stop=True,
                            )
                        m_sb = wk.tile([P, H], BF, tag="msb")
                        nc.scalar.activation(
                            out=m_sb[:], in_=m_ps[:],
                            func=mybir.ActivationFunctionType.Relu,
                        )
                        if _dbg and l == 0 and b == 0 and tt == 0:
                            nc.sync.dma_start(out=dbg_m[:, :], in_=m_sb[:])
                        # B_en one-hot [t, n]
                        ben = wk.tile([P, P], BF, tag="ben")
                        nc.vector.tensor_scalar(
                            out=ben[:], in0=iota[:],
                            scalar1=toffs[:, ti:ti + 1], scalar2=None,
                            op0=mybir.AluOpType.is_equal,
                        )
                        for k in range(2):
                            nc.tensor.matmul(
                                out=at_ps[k][:], lhsT=m_sb[:, k * P:(k + 1) * P],
                                rhs=ben[:],
                                start=(tt == 0), stop=(tt == nbt - 1),
                            )
                    t_t0 += nbt
                    # A_T psum -> sbuf, V_D = A_T.T @ W2b
                    at_sb = wk.tile([P, 2, P], BF, tag="atsb")
                    for k in range(2):
                        nc.scalar.activation(
                            out=at_sb[:, k, :], in_=at_ps[k][:],
                            func=mybir.ActivationFunctionType.Copy,
                        )
                    if _dbg and l == 0 and b == 0:
                        nc.sync.dma_start(
                            out=dbg_at[:, :],
                            in_=at_sb[:, :, :].rearrange("p k n -> p (k n)"))
                    vd_ps = ps1.tile([P, H], F32, space="PSUM", tag="misc")
                    for k in range(2):
                        nc.tensor.matmul(
                            out=vd_ps[:], lhsT=at_sb[:, k, :],
                            rhs=W[f"w2b_{l}"][:, k, :],
                            start=(k == 0), stop=(k == 1),
                        )
                    vd = wk.tile([P, H], BF, tag="vdsb")
                    nc.scalar.activation(
                        out=vd[:], in_=vd_ps[:],
                        func=mybir.ActivationFunctionType.Copy,
                    )
                    if _dbg and l == 0 and b == 0:
                        nc.sync.dma_start(out=dbg_vd[:, :], in_=vd[:])

                    # ---- edge stage ----
                    nbe = EBT[b]
                    nat_ps0 = ps1.tile([P, P], F32, space="PSUM", tag="nat0")
                    nat_ps1 = ps1.tile([P, P], F32, space="PSUM", tag="nat1")
                    nat_ps = [nat_ps0, nat_ps1]
                    for et in range(nbe):
                        ei = e_t0 + et
                        fin_ps = ps.tile([P, H], F32, space="PSUM", tag="work")
                        # A_en one-hot, then PE-transpose -> A_ne for V bcast
                        aen = wk.tile([P, P], BF, tag="aen")
                        nc.vector.tensor_scalar(
                            out=aen[:], in0=iota[:],
                            scalar1=eoffs[:, ei:ei + 1], scalar2=None,
                            op0=mybir.AluOpType.is_equal,
                        )
                        ane_ps = ps.tile([P, P], BF, space="PSUM", tag="work")
                        nc.tensor.transpose(
                            out=ane_ps[:], in_=aen[:], identity=idb[:])
                        ane = wk.tile([P, P], BF, tag="ane")
                        nc.scalar.activation(
                            out=ane[:], in_=ane_ps[:],
                            func=mybir.ActivationFunctionType.Copy,
                        )
                        nc.tensor.matmul(
                            out=fin_ps[:], lhsT=ane[:], rhs=vd[:],
                            start=True, stop=False,
                        )
                        if l == 0:
                            xs = wk.tile([P, kc_in, P], BF, tag="xs")
                            nc.sync.dma_start(
                                out=xs[:],
                                in_=d["e_xsrcT"][ei, :, :].rearrange(
                                    "(k p) e -> p k e", p=P),
                            )
                            for k in range(kc_in):
                                nc.tensor.matmul(
                                    out=fin_ps[:], lhsT=xs[:, k, :],
                                    rhs=W[f"w2t_{l}"][:, k, :],
                                    start=False, stop=(k == kc_in - 1),
                                )
                        else:
                            ug = wk.tile([P, H], BF, tag="ug")
                            nc.gpsimd.indirect_dma_start(
                                out=ug[:], out_offset=None, in_=table[:, :],
                                in_offset=bass.IndirectOffsetOnAxis(
                                    ap=esrc[:, ei:ei + 1], axis=0),
                                element_offset=0,
                            )
                            nc.tensor.matmul(
                                out=fin_ps[:], lhsT=idb[:], rhs=ug[:],
                                start=False, stop=True,
                            )
                        fin = wk.tile([P, H], BF, tag="finsb")
                        nc.scalar.activation(
                            out=fin[:], in_=fin_ps[:],
                            func=mybir.ActivationFunctionType.Relu,
                        )
                        if _dbg and l == 0 and b == 0 and et == 0:
                            nc.sync.dma_start(out=dbg_fin[:, :], in_=fin[:])
                        for k in range(2):
                            nc.tensor.matmul(
                                out=nat_ps[k][:],
                                lhsT=fin[:, k * P:(k + 1) * P], rhs=aen[:],
                                start=(et == 0), stop=(et == nbe - 1),
                            )
                    e_t0 += nbe

                    nat_sb = wk.tile([P, 2, P], BF, tag="natsb")
                    for k in range(2):
                        nc.scalar.activation(
                            out=nat_sb[:, k, :], in_=nat_ps[k][:],
                            func=mybir.ActivationFunctionType.Copy,
                        )
                    if _dbg and l == 0 and b == 0:
                        nc.sync.dma_start(
                            out=dbg_nat[:, :],
                            in_=nat_sb[:, :, :].rearrange("p k n -> p (k n)"))

                    # ---- node update ----
                    hn_ps = ps1.tile([P, H], F32, space="PSUM", tag="misc")
                    nc.tensor.matmul(
                        out=hn_ps[:], lhsT=ones_bf[:],
                        rhs=W[f"bn_{l}"][:], start=True, stop=False,
                    )
                    for k in range(kc_in):
                        nc.tensor.matmul(
                            out=hn_ps[:],
                            lhsT=hT[:, k, b * P:(b + 1) * P],
                            rhs=W[f"wn_{l}"][:, k, :],
                            start=False, stop=False,
                        )
                    for k in range(2):
                        nc.tensor.matmul(
                            out=hn_ps[:], lhsT=nat_sb[:, k, :],
                            rhs=W[f"wn_{l}"][:, kc_in + k, :],
                            start=False, stop=(k == 1),
                        )

                    if l < 2:
                        hn_sb = wk.tile([P, H], BF, tag="hnsb")
                        nc.scalar.activation(
                            out=hn_sb[:], in_=hn_ps[:],
                            func=mybir.ActivationFunctionType.Copy,
                        )
                        # transpose to maintain hT_new
                        for k in range(2):
                            tr_ps = ps1.tile([P, P], BF, space="PSUM", tag="misc")
                            nc.tensor.transpose(
                                out=tr_ps[:], in_=hn_sb[:, k * P:(k + 1) * P],
                                identity=idb[:],
                            )
                            nc.scalar.activation(
                                out=hT_new[:, k, b * P:(b + 1) * P], in_=tr_ps[:],
                                func=mybir.ActivationFunctionType.Copy,
                            )
                        # next-layer table rows: U = h' @ w2t, Hh = h' @ w1h
                        for half, wname in ((0, f"w2t_{l + 1}"), (1, f"w1h_{l + 1}")):
                            tb_ps = ps1.tile([P, H], F32, space="PSUM", tag="misc")
                            for k in range(2):
                                nc.tensor.matmul(
                                    out=tb_ps[:],
                                    lhsT=hT_new[:, k, b * P:(b + 1) * P],
                                    rhs=W[wname][:, k, :],
                                    start=(k == 0), stop=(k == 1),
                                )
                            tb_sb = wk.tile([P, H], BF, tag="tbsb")
                            nc.scalar.activation(
                                out=tb_sb[:], in_=tb_ps[:],
                                func=mybir.ActivationFunctionType.Copy,
                            )
                            nc.sync.dma_start(
                                out=ag_ins[l][b * P:(b + 1) * P,
                                              half * H:(half + 1) * H],
                                in_=tb_sb[:],
                            )
                    else:
                        # layer 3: pooling contribution
                        h3 = wk.tile([P, H], F32, tag="h3")
                        nc.scalar.activation(
                            out=h3[:], in_=hn_ps[:],
                            func=mybir.ActivationFunctionType.Copy,
                        )
                        pt = wk.tile([P, G], F32, tag="pt")
                        nc.sync.dma_start(out=pt[:], in_=d["poolT"][b, :, :])
                        nc.tensor.matmul(
                            out=pooled_ps[:], lhsT=pt[:], rhs=h3[:],
                            start=(b == 0), stop=(b == BLOCKS_PER_CORE - 1),
                        )

                if l < 2:
                    hT = hT_new
                    if _dbg and l == 0:
                        nc.sync.dma_start(
                            out=dbg_h2[:, :],
                            in_=hT_new[:, :, :].rearrange("p k n -> p (k n)"),
                        )
                    nc.gpsimd.collective_compute(
                        "AllGather",
                        mybir.AluOpType.bypass,
                        replica_groups=groups,
                        ins=[ag_ins[l][:, :]],
                        outs=[tables[l][:, :]],
                    )

            # ---- readout ----
            pooled = wk.tile([G, H], F32, tag="pooled_sb")
            nc.scalar.activation(
                out=pooled[:], in_=pooled_ps[:],
                func=mybir.ActivationFunctionType.Copy,
            )
            nc.sync.dma_start(out=ar_in[:, :], in_=pooled[:])
            nc.gpsimd.collective_compute(
                "AllReduce", mybir.AluOpType.add, replica_groups=groups,
                ins=[ar_in[:, :]], outs=[ar_out[:, :]],
            )
            gsum = wk.tile([G, H], F32, tag="gsum")
            nc.sync.dma_start(out=gsum[:], in_=ar_out[:, :])
            # g = relu(pooled)
            nc.vector.tensor_scalar_max(gsum[:], gsum[:], 0.0)

            idf = cs.tile([P, P], F32)
            nc.vector.tensor_copy(idf[:], idb[:])

            wo1 = cs.tile([P, 2, H], F32)
            nc.sync.dma_start(
                out=wo1[:], in_=d["wo1"][:, :, :].rearrange("k p h -> p k h"))
            bo1 = cs.tile([1, H], F32)
            nc.sync.dma_start(out=bo1[:], in_=d["bo1"][:, :])
            wo2 = cs.tile([P, 2, OUT_C], F32)
            nc.sync.dma_start(
                out=wo2[:], in_=d["wo2"][:, :, :].rearrange("k p h -> p k h"))
            bo2 = cs.tile([1, OUT_C], F32)
            nc.sync.dma_start(out=bo2[:], in_=d["bo2"][:, :])

            def transpose_gx(src_tile, width):
                # [G, width] f32 -> [P, width//P, G] f32 (lhsT chunks)
                out_t = wk.tile([P, width // P, G], F32, tag="gxT")
                for k in range(width // P):
                    tp = ps1.tile([P, G], F32, space="PSUM", tag="misc")
                    nc.tensor.transpose(
                        out=tp[:], in_=src_tile[:, k * P:(k + 1) * P],
                        identity=idf[0:G, 0:G],
                    )
                    nc.scalar.activation(
                        out=out_t[:, k, :], in_=tp[:],
                        func=mybir.ActivationFunctionType.Copy,
                    )
                return out_t

            gT = transpose_gx(gsum, H)
            o1_ps = ps1.tile([G, H], F32, space="PSUM", tag="misc")
            nc.tensor.matmul(out=o1_ps[:], lhsT=ones_f[:], rhs=bo1[:],
                             start=True, stop=False)
            for k in range(2):
                nc.tensor.matmul(out=o1_ps[:], lhsT=gT[:, k, :],
                                 rhs=wo1[:, k, :], start=False, stop=(k == 1))
            o1 = wk.tile([G, H], F32, tag="o1sb")
            nc.scalar.activation(out=o1[:], in_=o1_ps[:],
                                 func=mybir.ActivationFunctionType.Relu)
            o1T = transpose_gx(o1, H)
            o2_ps = ps1.tile([G, OUT_C], F32, space="PSUM", tag="misc")
            nc.tensor.matmul(out=o2_ps[:], lhsT=ones_f[:], rhs=bo2[:],
                             start=True, stop=False)
            for k in range(2):
                nc.tensor.matmul(out=o2_ps[:], lhsT=o1T[:, k, :],
                                 rhs=wo2[:, k, :], start=False, stop=(k == 1))
            o2 = wk.tile([G, OUT_C], F32, tag="o2sb")
            nc.scalar.activation(out=o2[:], in_=o2_ps[:],
                                 func=mybir.ActivationFunctionType.Copy)
            nc.sync.dma_start(out=out_ext[:, :], in_=o2[:])

    split_multi_waits(nc)
    return nc


_CACHE = {}


def kernel(**inputs) -> np.ndarray:
    plan = _plan(inputs)
    nc = _build(plan)
    wb = plan["weights"]
    in_maps = []
    for c in range(NCORES):
        m = dict(plan["per_core"][c])
        m = {
            "e_src": m["e_src"], "e_off": m["e_off"],
            "e_xsrcT": m["e_xsrcT"], "t_k": m["t_k"], "t_off": m["t_off"],
            "t_rcT": m["t_rcT"], "t_xkT": m["t_xkT"], "poolT": m["poolT"],
            "xT_own": m["xT_own"],
        }
        for k, v in wb.items():
            if k in ("iota_row", "ident_bf", "ones_bf", "ones_f32"):
                continue
            m[k] = v
        m["iota_row"] = wb["iota_row"]
        m["ident_bf"] = wb["ident_bf"]
        m["ones_bf"] = wb["ones_bf"]
        m["ones_f32"] = wb["ones_f32"]
        in_maps.append(m)
    res = run_bass_kernel_spmd(nc, in_maps, core_ids=list(range(NCORES)))
    return np.asarray(res.results[0]["out"], np.float32)


def kernel_profiled(**inputs):
    """Returns (out, exec_ns_estimate). Tries neuron-profile trace; falls back
    to steady-state wall time of repeated NEFF executions."""
    import time as _time
    plan = _plan(inputs)
    nc = _build(plan)
    wb = plan["weights"]
    in_maps = []
    for c in range(NCORES):
        m = dict(plan["per_core"][c])
        m.update(wb)
        in_maps.append(m)
    try:
        res = run_bass_kernel_spmd(
            nc, in_maps, core_ids=list(range(NCORES)), trace=True
        )
    except Exception:
        res = run_bass_kernel_spmd(nc, in_maps, core_ids=list(range(NCORES)))
    out = np.asarray(res.results[0]["out"], np.float32)
    if res.exec_time_ns is not None:
        return out, res.exec_time_ns, "neuron-profile"
    # fallback: repeated executes of the cached NEFF (includes dispatch)
    times = []
    for _ in range(3):
        t0 = _time.perf_counter()
        run_bass_kernel_spmd(nc, in_maps, core_ids=list(range(NCORES)))
        times.append(_time.perf_counter() - t0)
    return out, int(min(times) * 1e9), "wall-clock upper bound"


if __name__ == "__main__":
    pass
